# revision 44
# baseline (speedup 1.0000x reference)
"""Trainium2 Bass kernel for a dense transformer block (pre-LN, causal MHA, FFN).

Sharding: sequence-parallel over 8 cores. Each batch (B=2) is split into 4
query chunks of 512 tokens; core c handles batch c//4, chunk c%4. Each core
recomputes LN1+K/V over its causal context. To keep one SPMD program across
cores, the context is FRONT-PADDED with zeros to 2048 tokens so the query
chunk always sits at local positions [1536, 2048) -- the causal triangle is
then core-invariant, and padded keys are masked via a per-key additive bias
folded into the softmax Exp's per-partition bias operand (scores are computed
transposed, keys on partitions, so a per-key bias is a per-partition scalar).

Layout: activations are feature-major (E on partitions) so weight matrices
(stored (E_in, E_out)) serve directly as matmul lhsT tiles; weight chunks are
pre-tiled on the host so every chunk DMA is a single contiguous burst.
Transposed scores feed the P@V matmul without any transposes; the softmax
denominator comes from a ones column appended to V's lhsT (row DH of the
attnV psum). Softmax skips max-subtraction (scores are O(1) by construction;
a constant -5 shift is folded into the key bias).

Precision: the matmul datapath runs bf16 x bf16 -> fp32-psum (use_bf16=True;
fast weight loads + halved weight traffic). LN statistics, softmax
denominators, residuals and the final output stay fp32 (stats matmuls run as
float32r). With use_bf16=False the whole datapath runs float32r instead.
"""

import sys
from contextlib import ExitStack
from dataclasses import dataclass

import numpy as np

if "/opt/trn_rl_repo" not in sys.path:
    sys.path.insert(0, "/opt/trn_rl_repo")

import concourse.bass as bass  # noqa: E402
import concourse.mybir as mybir  # noqa: E402
import concourse.tile as tile  # noqa: E402
from concourse.vector_clock import ScopedClock  # noqa: E402

F32 = mybir.dt.float32
F32R = mybir.dt.float32r
BF16 = mybir.dt.bfloat16
AX = mybir.AluOpType
AF = mybir.ActivationFunctionType

MASK_NEG = -30000.0
EXP_SHIFT = -5.0


class TC(tile.TileContext):
    """TileContext whose kernel-tail drain splits its sem waits across
    separate SP instructions -- walrus in this env rejects >2 sync waits
    on one CTRL-class instruction -- and which post-splits any multi-wait
    instruction (the S3_LW fp32 matmul struct tolerates only one sync
    wait) by hoisting extra waits onto same-engine NoOps."""

    do_split_waits = True  # disable for CoreSim (breaks its fake-update bookkeeping)

    def schedule_and_allocate(self, *a, **k):
        ret = super().schedule_and_allocate(*a, **k)
        if self.do_split_waits:
            self._split_multiwaits()
        return ret

    def _split_multiwaits(self):
        import bass_rust
        n_new = 0
        for fn in self.nc.m.functions:
            for blk in fn.blocks:
                insts = list(blk.instructions)
                out = []
                changed = False
                for inst in insts:
                    si = inst.sync_info
                    waits = list(si.on_wait) if si is not None else []
                    if len(waits) > 1:
                        for w in waits[:-1]:
                            nop = mybir.InstNoOp(
                                name=f"{inst.name}-sw{n_new}", ins=[], outs=[])
                            nop.engine = inst.engine
                            nop.sync_info = bass_rust.SyncInfo(
                                on_wait=[w], on_update=[])
                            out.append(nop)
                            n_new += 1
                        si.on_wait = [waits[-1]]
                        changed = True
                    out.append(inst)
                if changed:
                    blk.instructions = out

    def _drain_and_barrier(self, tick_clock, wait_clock):
        probe = self.nc.sync.nop(nofuse=True)
        wait_clock.add_sem_waits(probe.ins, ScopedClock({None: tick_clock.global_clock}))
        waits = list(probe.ins.sync_info.on_wait)
        assert self.sems is not None
        alloc = self.sems.allocated()
        by_name = {getattr(h, "name", k): h for k, h in alloc.items()}
        if len(waits) > 1:
            probe.ins.sync_info.on_wait = [waits[0]]
            for w in waits[1:]:
                self.nc.sync.wait_ge(by_name[w.ant_name], w.wait_value)
        self.nc.sync.drain()
        self.nc.all_engine_barrier()
        popped = self.nc._tile_sem_poison_stack.pop()
        assert popped is self._sem_poison
        self.nc.clear_and_free_semaphores(list(alloc.values()))
        self.nc.all_engine_barrier()


@dataclass(frozen=True)
class Cfg:
    P: int = 128          # partitions
    E: int = 1024         # embed dim
    H: int = 16           # heads
    DH: int = 64          # head dim
    HID: int = 4096       # ffn hidden
    CTX: int = 2048       # padded context length per core
    TQ: int = 512         # query tokens per core
    eps: float = 1e-5
    n_cores: int = 8
    n_vblocks: int = 1    # V/attention processed in this many head blocks
    use_bf16: bool = True

    @property
    def ET(self):
        return self.E // self.P

    @property
    def JT(self):
        return self.CTX // self.P

    @property
    def TT(self):
        return self.CTX // self.P

    @property
    def G(self):
        return self.CTX // self.TQ

    @property
    def HOT(self):
        return self.HID // self.P

    @property
    def HB(self):  # heads per V block
        return self.H // self.n_vblocks


def f32r(ap):
    return ap.bitcast(F32R)


def build_program(cfg: Cfg, split_waits: bool = True) -> bass.Bass:
    P, E, H, DH, HID = cfg.P, cfg.E, cfg.H, cfg.DH, cfg.HID
    CTX, TQ, ET, JT, G, HOT, HB, TT = (
        cfg.CTX, cfg.TQ, cfg.ET, cfg.JT, cfg.G, cfg.HOT, cfg.HB, cfg.TT)
    NVB = cfg.n_vblocks
    NG = TQ
    assert H == 2 * ET and DH * H == E and DH * 2 == P
    assert HB >= 2 and ET % NVB == 0
    VW = HB * DH  # V-projection moving width
    VH = max(1, VW // 512)   # psum output is capped at 512 fp32 columns
    VWH = VW // VH
    assert VWH <= 512

    DT = BF16 if cfg.use_bf16 else F32

    def rnd(ap):
        """Matmul-operand producer/consumer wrapper for the main datapath."""
        return ap if cfg.use_bf16 else ap.bitcast(F32R)

    nc = bass.Bass("TRN2", num_devices=cfg.n_cores)

    xdev = nc.declare_dram_parameter("xdev", [P, ET, CTX], F32, isOutput=False)
    # pre-tiled weight chunks (host layout): contiguous per chunk
    Wqc = nc.declare_dram_parameter("Wqc", [ET, P, ET, P], DT, isOutput=False)
    Wkc = nc.declare_dram_parameter("Wkc", [ET, P, ET, P], DT, isOutput=False)
    Wvc = nc.declare_dram_parameter("Wvc", [NVB, P, ET, VW], DT, isOutput=False)
    Woc = nc.declare_dram_parameter("Woc", [ET, P, ET, P], DT, isOutput=False)
    W1c = nc.declare_dram_parameter("W1c", [HOT, P, ET, P], DT, isOutput=False)
    W2t = nc.declare_dram_parameter("W2t", [HOT, P, E], DT, isOutput=False)
    lnw1 = nc.declare_dram_parameter("lnw1", [P, ET], F32, isOutput=False)
    lnb1 = nc.declare_dram_parameter("lnb1", [P, ET], F32, isOutput=False)
    lnw2 = nc.declare_dram_parameter("lnw2", [P, ET], F32, isOutput=False)
    lnb2 = nc.declare_dram_parameter("lnb2", [P, ET], F32, isOutput=False)
    bod = nc.declare_dram_parameter("bo", [P, ET], F32, isOutput=False)
    b1d = nc.declare_dram_parameter("b1", [P, HOT], F32, isOutput=False)
    b2d = nc.declare_dram_parameter("b2", [P, ET], F32, isOutput=False)
    kbd = nc.declare_dram_parameter("kb", [P, JT], F32, isOutput=False)
    outT = nc.declare_dram_parameter("outT", [P, ET, TQ], F32, isOutput=True)

    scale = 1.0 / float(np.sqrt(DH))
    j_tri0 = (CTX - TQ) // P
    qs = slice(CTX - TQ, CTX)

    _ones_row = []  # (1, P) f32r-rounded ones, set up in the const section

    def bcast(ps_pool, tag, nparts, row):
        """Broadcast a (1, n) f32r SBUF row across nparts partitions via a
        PE outer product (ones[1,nparts].T @ row) into a PSUM tile."""
        ps_b = ps_pool.tile([nparts, row.shape[-1]], F32, tag=tag, name=f"bc_{tag}")
        nc.tensor.matmul(ps_b, f32r(_ones_row[0][:, 0:nparts]), f32r(row),
                         start=True, stop=True)
        return ps_b

    def ln_stats(rows_p, pbc_p, ps_sum, ps_sq, eps_row):
        """psum sums -> (nmean_b, rstd_b) PSUM broadcast tiles."""
        n = ps_sum.shape[-1]
        nmean = rows_p.tile([1, n], F32, tag="rows")
        nc.vector.tensor_scalar_mul(f32r(nmean), ps_sum, -1.0 / E)
        msq = rows_p.tile([1, n], F32, tag="rows")
        nc.vector.tensor_mul(msq, nmean, nmean)
        var = rows_p.tile([1, n], F32, tag="rows")
        nc.vector.scalar_tensor_tensor(
            out=var, in0=ps_sq, scalar=1.0 / E, in1=msq,
            op0=AX.mult, op1=AX.subtract)
        sq = rows_p.tile([1, n], F32, tag="rows")
        nc.scalar.activation(out=sq, in_=var, func=AF.Sqrt, bias=eps_row)
        rstd = rows_p.tile([1, n], F32, tag="rows")
        nc.vector.reciprocal(f32r(rstd), sq)
        nmean_b = bcast(pbc_p, "pbc", P, nmean)
        rstd_b = bcast(pbc_p, "pbc", P, rstd)
        return nmean_b, rstd_b

    def ln_apply(tmp_p, dst, src, nmean_b, rstd_b, w_col, b_col):
        """dst = LN(src)*w + b; intermediates in fp32, final write casts."""
        t = tmp_p.tile([P, dst.shape[-1]], F32, tag="lnt")
        nc.vector.tensor_add(t, src, nmean_b)
        nc.vector.scalar_tensor_tensor(
            out=t, in0=t, scalar=w_col, in1=rstd_b,
            op0=AX.mult, op1=AX.mult)
        nc.vector.tensor_scalar_add(rnd(dst), t, b_col)

    with TC(nc, num_cores=cfg.n_cores) as tc, \
            nc.allow_low_precision(reason="reduced-precision matmul datapath"):
        tc.do_split_waits = split_waits
        with ExitStack() as top:
            const_p = top.enter_context(tc.tile_pool(name="consts", bufs=1))
            ht_p = top.enter_context(tc.tile_pool(name="ht", bufs=1))

            ones = const_p.tile([P, 1], F32)
            nc.vector.memset(ones, 1.0)
            ones_r = const_p.tile([P, 1], F32)
            nc.vector.tensor_copy(f32r(ones_r), ones)
            ones_hb = const_p.tile([P, HB, 1], F32)
            nc.vector.memset(ones_hb, 1.0)
            ones_row = const_p.tile([1, P], F32)
            nc.vector.memset(ones_row, 1.0)
            ones_row_r = const_p.tile([1, P], F32)
            nc.vector.tensor_copy(f32r(ones_row_r), ones_row)
            _ones_row.append(ones_row_r)
            eps_row = const_p.tile([1, 1], F32)
            nc.vector.memset(eps_row, cfg.eps)
            KB = const_p.tile([P, JT], F32)
            nc.sync.dma_start(out=KB, in_=kbd[:])
            LNW1 = const_p.tile([P, ET], F32)
            nc.sync.dma_start(out=LNW1, in_=lnw1[:])
            LNB1 = const_p.tile([P, ET], F32)
            nc.sync.dma_start(out=LNB1, in_=lnb1[:])
            LNW2 = const_p.tile([P, ET], F32)
            nc.sync.dma_start(out=LNW2, in_=lnw2[:])
            LNB2 = const_p.tile([P, ET], F32)
            nc.sync.dma_start(out=LNB2, in_=lnb2[:])
            BO = const_p.tile([P, ET], F32)
            nc.sync.dma_start(out=BO, in_=bod[:])
            B1 = const_p.tile([P, HOT], F32)
            nc.sync.dma_start(out=B1, in_=b1d[:])
            B2 = const_p.tile([P, ET], F32)
            nc.sync.dma_start(out=B2, in_=b2d[:])

            # causal-triangle multiplicative masks for key tiles j_tri0..JT-1
            n_tri = JT - j_tri0
            TRI = const_p.tile([P, n_tri, TQ], DT)
            with tc.tile_pool(name="trisc", bufs=2) as tri_p:
                for jj in range(n_tri):
                    tsc = tri_p.tile([P, TQ], F32, tag="trisc")
                    nc.vector.memset(tsc, 1.0)
                    base = (CTX - TQ) - (j_tri0 + jj) * P
                    nc.gpsimd.affine_select(
                        out=tsc, in_=tsc, compare_op=AX.is_ge, fill=0.0,
                        base=base, pattern=[[1, TQ]], channel_multiplier=-1)
                    nc.vector.tensor_copy(rnd(TRI[:, jj, :]), tsc)

            with ExitStack() as mid:
                with ExitStack() as attn_sc:
                    xn_p = attn_sc.enter_context(tc.tile_pool(name="xn", bufs=1))
                    qt_p = attn_sc.enter_context(tc.tile_pool(name="qt", bufs=1))
                    va_p = attn_sc.enter_context(tc.tile_pool(name="va", bufs=1))
                    at_p = attn_sc.enter_context(tc.tile_pool(name="at", bufs=1))
                    xq_p = attn_sc.enter_context(tc.tile_pool(name="xq", bufs=1))
                    wo_p = attn_sc.enter_context(tc.tile_pool(name="wo", bufs=ET))
                    XN = xn_p.tile([P, ET, CTX], DT)
                    QT = qt_p.tile([P, ET, TQ], DT)
                    VA = va_p.tile([P, TT, HB, DH + 1], DT)
                    assert NVB == 1

                    # -------- phase A: LN1 + Q-proj + V-proj overlapped -----
                    with tc.tile_pool(name="xs", bufs=ET + 2) as xs_p, \
                         tc.tile_pool(name="xsq", bufs=3) as xsq_p, \
                         tc.tile_pool(name="lnt", bufs=3) as lnt_p, \
                         tc.tile_pool(name="rows", bufs=6) as rows_p, \
                         tc.tile_pool(name="wcq", bufs=4) as wcq_p, \
                         tc.tile_pool(name="wv", bufs=1) as wv_p, \
                         tc.tile_pool(name="pstat", bufs=3, space="PSUM") as pstat_p, \
                         tc.tile_pool(name="pbc", bufs=2, space="PSUM") as pbc_p, \
                         tc.tile_pool(name="ppv", bufs=2, space="PSUM") as ppv_p:

                        def ln_group(g):
                            gs = slice(g * NG, (g + 1) * NG)
                            ps_sum = pstat_p.tile([1, NG], F32, tag="pstat",
                                                  name=f"pssum{g}")
                            ps_sq = pstat_p.tile([1, NG], F32, tag="pstat",
                                                 name=f"pssq{g}")
                            xs_tiles = []
                            for et in range(ET):
                                xs = xs_p.tile([P, NG], F32, tag="xs")
                                nc.sync.dma_start(out=f32r(xs),
                                                  in_=f32r(xdev[:, et, gs]))
                                xs_tiles.append(xs)
                                xsq = xsq_p.tile([P, NG], F32, tag="xsq")
                                nc.scalar.square(out=f32r(xsq), in_=xs)
                                nc.tensor.matmul(ps_sum, f32r(ones_r), f32r(xs),
                                                 start=(et == 0), stop=(et == ET - 1))
                                nc.tensor.matmul(ps_sq, f32r(ones_r), f32r(xsq),
                                                 start=(et == 0), stop=(et == ET - 1))
                            nmean_b, rstd_b = ln_stats(rows_p, pbc_p, ps_sum, ps_sq,
                                                       eps_row)
                            for et in range(ET):
                                ln_apply(lnt_p, XN[:, et, gs], xs_tiles[et],
                                         nmean_b, rstd_b,
                                         LNW1[:, et:et + 1], LNB1[:, et:et + 1])

                        ln_group(G - 1)

                        # Q projection (depends only on the last group)
                        for eo in range(ET):
                            wq = wcq_p.tile([P, ET, P], DT, tag="wcq")
                            nc.sync.dma_start(out=rnd(wq), in_=rnd(Wqc[eo]))
                            ps = ppv_p.tile([P, TQ], F32, tag="ppv",
                                            name=f"psq{eo}")
                            for et in range(ET):
                                nc.tensor.matmul(ps, rnd(wq[:, et, :]),
                                                 rnd(XN[:, et, qs]),
                                                 start=(et == 0), stop=(et == ET - 1))
                            nc.vector.tensor_copy(rnd(QT[:, eo, :]), ps)

                        wv = wv_p.tile([P, ET, VW], DT)
                        nc.sync.dma_start(out=rnd(wv), in_=rnd(Wvc[0]))

                        for g in range(G - 1):
                            ln_group(g)

                        # V projection (token-major, ones column appended)
                        hh_per = VWH // DH
                        for tt in range(TT):
                            nc.vector.tensor_copy(rnd(VA[:, tt, :, DH:DH + 1]),
                                                  ones_hb)
                            for vh in range(VH):
                                ps = ppv_p.tile([P, VWH], F32, tag="ppv",
                                                name=f"psv{tt}_{vh}")
                                for et in range(ET):
                                    nc.tensor.matmul(
                                        ps, rnd(XN[:, et, tt * P:(tt + 1) * P]),
                                        rnd(wv[:, et, vh * VWH:(vh + 1) * VWH]),
                                        start=(et == 0), stop=(et == ET - 1))
                                nc.vector.tensor_copy(
                                    rnd(VA[:, tt, vh * hh_per:(vh + 1) * hh_per,
                                            0:DH]),
                                    ps.rearrange("p (h d) -> p h d", d=DH))

                    # -------- phase C: K-proj + attention + outproj --------
                    AT = at_p.tile([P, ET, TQ], DT)
                    HT = ht_p.tile([P, ET, TQ], F32)
                    XQ = xq_p.tile([P, ET, TQ], F32)
                    for et in range(ET):
                        nc.sync.dma_start(out=XQ[:, et, :], in_=xdev[:, et, qs])
                    wo_tiles = []
                    for eo in range(ET):
                        wo = wo_p.tile([P, ET, P], DT, tag="wo")
                        nc.sync.dma_start(out=rnd(wo), in_=rnd(Woc[eo]))
                        wo_tiles.append(wo)
                    with tc.tile_pool(name="wc", bufs=ET) as wc_p, \
                         tc.tile_pool(name="kt", bufs=3) as kt_p, \
                         tc.tile_pool(name="pt", bufs=4) as pt_p, \
                         tc.tile_pool(name="arow", bufs=4) as arow_p, \
                         tc.tile_pool(name="avs", bufs=4) as avs_p, \
                         tc.tile_pool(name="pproj", bufs=2, space="PSUM") as pproj_p, \
                         tc.tile_pool(name="psc", bufs=2, space="PSUM") as psc_p, \
                         tc.tile_pool(name="pav", bufs=2, space="PSUM") as pav_p:

                        pending_norm = []

                        def flush_norm():
                            while pending_norm:
                                av, h = pending_norm.pop(0)
                                rr = arow_p.tile([1, TQ], F32, tag="arow",
                                                 name=f"rr{h}")
                                nc.vector.reciprocal(f32r(rr), av[DH:DH + 1, :])
                                rb_ps = bcast(psc_p, "psc", DH, rr)
                                dst = AT[(h % 2) * DH:((h % 2) + 1) * DH,
                                         h // 2, :]
                                nc.vector.tensor_mul(rnd(dst), av[0:DH, :], rb_ps)

                        wk_tiles = []
                        for eo in range(ET):
                            wk = wc_p.tile([P, ET, P], DT, tag="wc")
                            nc.sync.dma_start(out=rnd(wk), in_=rnd(Wkc[eo]))
                            wk_tiles.append(wk)
                        for eo in range(ET):
                            hA, hB = 2 * eo, 2 * eo + 1
                            # K^T projection for heads hA, hB
                            kt = kt_p.tile([P, CTX], DT, tag="kt")
                            wk = wk_tiles[eo]
                            for g in range(G):
                                gs2 = slice(g * NG, (g + 1) * NG)
                                ps = pproj_p.tile([P, NG], F32, tag="pproj")
                                for et in range(ET):
                                    nc.tensor.matmul(
                                        ps, rnd(wk[:, et, :]), rnd(XN[:, et, gs2]),
                                        start=(et == 0), stop=(et == ET - 1))
                                nc.vector.tensor_copy(rnd(kt[:, gs2]), ps)

                            ps_avA = pav_p.tile([P, TQ], F32, tag="pav")
                            ps_avB = pav_p.tile([P, TQ], F32, tag="pav")
                            pts = {}

                            def escore(j):
                                js = slice(j * P, (j + 1) * P)
                                psc = psc_p.tile([P, 2, TQ], F32, tag="psc")
                                nc.tensor.matmul(
                                    psc[:, 0, :], rnd(kt[0:DH, js]),
                                    rnd(QT[0:DH, eo, :]), start=True, stop=True)
                                nc.tensor.matmul(
                                    psc[:, 1, :], rnd(kt[DH:P, js]),
                                    rnd(QT[DH:P, eo, :]), start=True, stop=True)
                                pt = pt_p.tile([P, 2, TQ], DT, tag="pt")
                                nc.scalar.activation(
                                    out=rnd(pt), in_=psc, func=AF.Exp,
                                    bias=KB[:, j:j + 1], scale=scale)
                                if j >= j_tri0:
                                    m = TRI[:, j - j_tri0, :]
                                    mb = bass.AP(
                                        tensor=m.tensor, offset=m.offset,
                                        ap=[list(m.ap[0]), [0, 2], list(m.ap[1])])
                                    nc.vector.tensor_mul(rnd(pt), pt, mb)
                                pts[j] = pt

                            def eav(j):
                                pt = pts.pop(j)
                                nc.tensor.matmul(
                                    ps_avA[0:DH + 1, :], rnd(VA[:, j, hA, :]),
                                    rnd(pt[:, 0, :]),
                                    start=(j == 0), stop=(j == JT - 1))
                                nc.tensor.matmul(
                                    ps_avB[0:DH + 1, :], rnd(VA[:, j, hB, :]),
                                    rnd(pt[:, 1, :]),
                                    start=(j == 0), stop=(j == JT - 1))

                            escore(0)
                            for j in range(1, JT):
                                escore(j)
                                eav(j - 1)
                            eav(JT - 1)
                            avA = avs_p.tile([DH + 1, TQ], F32, tag="avs")
                            nc.vector.tensor_copy(avA, ps_avA[0:DH + 1, :])
                            avB = avs_p.tile([DH + 1, TQ], F32, tag="avs")
                            nc.vector.tensor_copy(avB, ps_avB[0:DH + 1, :])
                            flush_norm()
                            pending_norm += [(avA, hA), (avB, hB)]
                        flush_norm()

                        # out-projection + residual -> HT
                        for eo in range(ET):
                            ps = pproj_p.tile([P, TQ], F32, tag="pproj",
                                              name=f"pso{eo}")
                            for et in range(ET):
                                nc.tensor.matmul(ps, rnd(wo_tiles[eo][:, et, :]),
                                                 rnd(AT[:, et, :]),
                                                 start=(et == 0), stop=(et == ET - 1))
                            dst = HT[:, eo, :]
                            nc.vector.tensor_add(f32r(dst), ps, XQ[:, eo, :])
                            nc.vector.tensor_scalar_add(f32r(dst), dst,
                                                        BO[:, eo:eo + 1])

                # -------- LN2 --------
                lt_p = mid.enter_context(tc.tile_pool(name="lt", bufs=1))
                rt_p = mid.enter_context(tc.tile_pool(name="rt", bufs=1))
                LT = lt_p.tile([P, ET, TQ], DT)
                RT = rt_p.tile([P, HOT, TQ], DT)
                with tc.tile_pool(name="lnt2", bufs=3) as lnt2_p, \
                     tc.tile_pool(name="sq2", bufs=3) as sq2_p, \
                     tc.tile_pool(name="rows2", bufs=6) as rows2_p, \
                     tc.tile_pool(name="pstat2", bufs=2, space="PSUM") as pstat2_p, \
                     tc.tile_pool(name="pbc2", bufs=2, space="PSUM") as pbc2_p:
                    ps_sum = pstat2_p.tile([1, TQ], F32, tag="pstat2", name="l2sum")
                    ps_sq = pstat2_p.tile([1, TQ], F32, tag="pstat2", name="l2sq")
                    for et in range(ET):
                        hsq = sq2_p.tile([P, TQ], F32, tag="sq2")
                        nc.scalar.square(out=f32r(hsq), in_=HT[:, et, :])
                        nc.tensor.matmul(ps_sum, f32r(ones_r),
                                         f32r(HT[:, et, :]),
                                         start=(et == 0), stop=(et == ET - 1))
                        nc.tensor.matmul(ps_sq, f32r(ones_r), f32r(hsq),
                                         start=(et == 0), stop=(et == ET - 1))
                    nmean_b, rstd_b = ln_stats(rows2_p, pbc2_p, ps_sum, ps_sq,
                                               eps_row)
                    for et in range(ET):
                        ln_apply(lnt2_p, LT[:, et, :], HT[:, et, :],
                                 nmean_b, rstd_b,
                                 LNW2[:, et:et + 1], LNB2[:, et:et + 1])

                # -------- FFN1 + FFN2 first half (pipelined per ho) --------
                EH = ET // 2
                w2br_p = mid.enter_context(tc.tile_pool(name="w2br", bufs=1))
                W2BR = w2br_p.tile([P, HOT, E - EH * P], DT)
                with tc.tile_pool(name="w1", bufs=6) as w1_p, \
                     tc.tile_pool(name="w2a", bufs=4) as w2a_p, \
                     tc.tile_pool(name="ot", bufs=3) as ot_p, \
                     tc.tile_pool(name="pf2a", bufs=EH, space="PSUM") as pf2a_p:
                    pf1_ctx = ExitStack()
                    pf1_p = pf1_ctx.enter_context(
                        tc.tile_pool(name="pf1", bufs=3, space="PSUM"))
                    ps8a = [pf2a_p.tile([P, TQ], F32, tag="pf2a", name=f"ps8a_{i}")
                            for i in range(EH)]
                    def effn1(ho):
                        w1s = w1_p.tile([P, ET, P], DT, tag="w1")
                        nc.sync.dma_start(out=rnd(w1s), in_=rnd(W1c[ho]))
                        ps = pf1_p.tile([P, TQ], F32, tag="pf1", name=f"psf{ho}")
                        for et in range(ET):
                            nc.tensor.matmul(ps, rnd(w1s[:, et, :]),
                                             rnd(LT[:, et, :]),
                                             start=(et == 0), stop=(et == ET - 1))
                        nc.scalar.activation(out=rnd(RT[:, ho, :]), in_=ps,
                                             func=AF.Relu, bias=B1[:, ho:ho + 1])

                    def effn2a(ho):
                        nc.sync.dma_start(out=rnd(W2BR[:, ho, :]),
                                          in_=rnd(W2t[ho, :, EH * P:E]))
                        w2a = w2a_p.tile([P, EH * P], DT, tag="w2a")
                        nc.sync.dma_start(out=rnd(w2a),
                                          in_=rnd(W2t[ho, :, 0:EH * P]))
                        for eo in range(EH):
                            nc.tensor.matmul(
                                ps8a[eo], rnd(w2a[:, eo * P:(eo + 1) * P]),
                                rnd(RT[:, ho, :]),
                                start=(ho == 0), stop=(ho == HOT - 1))

                    effn1(0)
                    for ho in range(1, HOT):
                        effn1(ho)
                        effn2a(ho - 1)
                    effn2a(HOT - 1)
                    pf1_ctx.close()

                    # -------- FFN2 second half (pf1 banks recycled) --------
                    with tc.tile_pool(name="ot2", bufs=3) as ot2_p, \
                         tc.tile_pool(name="pf2b", bufs=ET - EH,
                                      space="PSUM") as pf2b_p:
                        ps8b = [pf2b_p.tile([P, TQ], F32, tag="pf2b",
                                            name=f"ps8b_{i}")
                                for i in range(ET - EH)]
                        first = True
                        for ho in range(HOT):
                            for eo in range(EH, ET):
                                nc.tensor.matmul(
                                    ps8b[eo - EH],
                                    rnd(W2BR[:, ho,
                                             (eo - EH) * P:(eo - EH + 1) * P]),
                                    rnd(RT[:, ho, :]),
                                    start=(ho == 0), stop=(ho == HOT - 1))
                            if first:
                                # drain first-half outputs behind 2b matmuls
                                first = False
                                for eo in range(EH):
                                    o = ot_p.tile([P, TQ], F32, tag="ot")
                                    nc.vector.tensor_add(o, ps8a[eo],
                                                         HT[:, eo, :])
                                    nc.vector.tensor_scalar_add(
                                        o, o, B2[:, eo:eo + 1])
                                    nc.sync.dma_start(out=outT[:, eo, :],
                                                      in_=o)
                        for eo in range(EH, ET):
                            o = ot2_p.tile([P, TQ], F32, tag="ot2")
                            nc.vector.tensor_add(o, ps8b[eo - EH],
                                                 HT[:, eo, :])
                            nc.vector.tensor_scalar_add(o, o, B2[:, eo:eo + 1])
                            nc.sync.dma_start(out=outT[:, eo, :], in_=o)
    return nc


# ------------------------- host side -------------------------

def _np_dt(cfg: Cfg):
    if cfg.use_bf16:
        import ml_dtypes
        return ml_dtypes.bfloat16
    return np.float32


def make_weight_inputs(cfg: Cfg, Wq, Wk, Wv, Wo, W1, W2):
    """Pre-tile weights into contiguous chunk layouts (shared by all cores)."""
    P, E, HID, ET, HOT, NVB, HB, DH = (
        cfg.P, cfg.E, cfg.HID, cfg.ET, cfg.HOT, cfg.n_vblocks, cfg.HB, cfg.DH)
    VW = HB * DH
    dt = _np_dt(cfg)

    def chunks_pp(W):  # (E,E) -> (ET_out, P, ET_in, P) slabs
        W = np.asarray(W, dtype=np.float32)
        # slab[eo, p, et, j] = W[et*P+p, eo*P+j]
        c = W.reshape(ET, P, ET, P).transpose(2, 1, 0, 3)
        return np.ascontiguousarray(c.astype(dt))

    Wv = np.asarray(Wv, dtype=np.float32)
    wvc = Wv.reshape(ET, P, NVB, VW).transpose(2, 1, 0, 3)
    W1 = np.asarray(W1, dtype=np.float32)
    w1c = W1.reshape(ET, P, HOT, P).transpose(2, 1, 0, 3)
    W2 = np.asarray(W2, dtype=np.float32)
    w2t = W2.reshape(HOT, P, E)

    return {
        "Wqc": chunks_pp(Wq),
        "Wkc": chunks_pp(Wk),
        "Woc": chunks_pp(Wo),
        "Wvc": np.ascontiguousarray(wvc.astype(dt)),
        "W1c": np.ascontiguousarray(w1c.astype(dt)),
        "W2t": np.ascontiguousarray(w2t.astype(dt)),
    }


def make_core_inputs(cfg: Cfg, core: int, weight_inputs, x, bo, ln1_w, ln1_b,
                     ln2_w, ln2_b, b1, b2):
    P, E, CTX, TQ, ET, JT, HOT = (
        cfg.P, cfg.E, cfg.CTX, cfg.TQ, cfg.ET, cfg.JT, cfg.HOT)
    n_chunks = CTX // TQ
    b, ci = core // n_chunks, core % n_chunks
    ctx_len = (ci + 1) * TQ
    pad = CTX - ctx_len

    xT = np.zeros((E, CTX), dtype=np.float32)
    xT[:, pad:] = np.asarray(x[b, :ctx_len], dtype=np.float32).T
    xdev = np.ascontiguousarray(xT.reshape(ET, P, CTX).transpose(1, 0, 2))

    kb = np.where(np.arange(CTX) < pad, np.float32(MASK_NEG), np.float32(0.0))
    kb = (kb + np.float32(EXP_SHIFT)).astype(np.float32)
    kb2d = np.ascontiguousarray(kb.reshape(JT, P).T)

    def cols(v, nt):
        return np.ascontiguousarray(
            np.asarray(v, dtype=np.float32).reshape(nt, P).T)

    m = {
        "xdev": xdev,
        "lnw1": cols(ln1_w, ET), "lnb1": cols(ln1_b, ET),
        "lnw2": cols(ln2_w, ET), "lnb2": cols(ln2_b, ET),
        "bo": cols(bo, ET), "b1": cols(b1, HOT), "b2": cols(b2, ET),
        "kb": kb2d,
    }
    m.update(weight_inputs)
    return m


def make_all_core_inputs(cfg: Cfg, **inputs):
    w = make_weight_inputs(cfg, inputs["Wq"], inputs["Wk"], inputs["Wv"],
                           inputs["Wo"], inputs["W1"], inputs["W2"])
    rest = {k: inputs[k] for k in
            ("x", "bo", "ln1_w", "ln1_b", "ln2_w", "ln2_b", "b1", "b2")}
    return [make_core_inputs(cfg, c, w, **rest) for c in range(cfg.n_cores)]


def unshard_output(cfg: Cfg, results):
    """results: list of per-core dicts with 'outT' -> full (B, S, E)."""
    P, E, TQ, ET, CTX = cfg.P, cfg.E, cfg.TQ, cfg.ET, cfg.CTX
    n_chunks = CTX // TQ
    B = cfg.n_cores // n_chunks
    S = n_chunks * TQ
    out = np.empty((B, S, E), dtype=np.float32)
    for core in range(cfg.n_cores):
        b, ci = core // n_chunks, core % n_chunks
        oT = results[core]["outT"]  # (P, ET, TQ)
        out[b, ci * TQ:(ci + 1) * TQ, :] = (
            oT.transpose(1, 0, 2).reshape(E, TQ).T)
    return out


_CACHE = {}


def _get_program(cfg: Cfg) -> bass.Bass:
    if cfg not in _CACHE:
        _CACHE[cfg] = build_program(cfg)
    return _CACHE[cfg]


def kernel(**inputs) -> np.ndarray:
    from concourse.bass_utils import run_bass_kernel_spmd
    cfg = Cfg()
    nc = _get_program(cfg)
    in_maps = make_all_core_inputs(cfg, **inputs)
    res = run_bass_kernel_spmd(nc, in_maps, list(range(cfg.n_cores)))
    return unshard_output(cfg, res.results)



# revision 45
# speedup vs baseline: 1.1937x; 1.1937x over previous
"""Trainium2 Bass kernel for a dense transformer block (pre-LN, causal MHA, FFN).

Sharding: sequence-parallel over 8 cores. Each batch (B=2) is split into 4
query chunks of 512 tokens; core c handles batch c//4, chunk c%4. Each core
recomputes LN1+K/V over its causal context. To keep one SPMD program across
cores, the context is FRONT-PADDED with zeros to 2048 tokens so the query
chunk always sits at local positions [1536, 2048) -- the causal triangle is
then core-invariant, and padded keys are masked via a per-key additive bias
folded into the softmax Exp's per-partition bias operand (scores are computed
transposed, keys on partitions, so a per-key bias is a per-partition scalar).

Layout: activations are feature-major (E on partitions) so weight matrices
(stored (E_in, E_out)) serve directly as matmul lhsT tiles; weight chunks are
pre-tiled on the host so every chunk DMA is a single contiguous burst.
Transposed scores feed the P@V matmul without any transposes; the softmax
denominator comes from a ones column appended to V's lhsT (row DH of the
attnV psum). Softmax skips max-subtraction (scores are O(1) by construction;
a constant -5 shift is folded into the key bias).

Precision: the matmul datapath runs bf16 x bf16 -> fp32-psum (use_bf16=True;
fast weight loads + halved weight traffic). LN statistics, softmax
denominators, residuals and the final output stay fp32 (stats matmuls run as
float32r). With use_bf16=False the whole datapath runs float32r instead.
"""

import sys
from contextlib import ExitStack
from dataclasses import dataclass

import numpy as np

if "/opt/trn_rl_repo" not in sys.path:
    sys.path.insert(0, "/opt/trn_rl_repo")

import concourse.bass as bass  # noqa: E402
import concourse.mybir as mybir  # noqa: E402
import concourse.tile as tile  # noqa: E402
from concourse.vector_clock import ScopedClock  # noqa: E402

F32 = mybir.dt.float32
F32R = mybir.dt.float32r
BF16 = mybir.dt.bfloat16
AX = mybir.AluOpType
AF = mybir.ActivationFunctionType

MASK_NEG = -30000.0
EXP_SHIFT = -5.0


class TC(tile.TileContext):
    """TileContext whose kernel-tail drain splits its sem waits across
    separate SP instructions -- walrus in this env rejects >2 sync waits
    on one CTRL-class instruction -- and which post-splits any multi-wait
    instruction (the S3_LW fp32 matmul struct tolerates only one sync
    wait) by hoisting extra waits onto same-engine NoOps."""

    do_split_waits = True  # disable for CoreSim (breaks its fake-update bookkeeping)

    def schedule_and_allocate(self, *a, **k):
        ret = super().schedule_and_allocate(*a, **k)
        if self.do_split_waits:
            self._split_multiwaits()
        return ret

    def _split_multiwaits(self):
        import bass_rust
        n_new = 0
        for fn in self.nc.m.functions:
            for blk in fn.blocks:
                insts = list(blk.instructions)
                out = []
                changed = False
                for inst in insts:
                    si = inst.sync_info
                    waits = list(si.on_wait) if si is not None else []
                    if len(waits) > 1:
                        for w in waits[:-1]:
                            nop = mybir.InstNoOp(
                                name=f"{inst.name}-sw{n_new}", ins=[], outs=[])
                            nop.engine = inst.engine
                            nop.sync_info = bass_rust.SyncInfo(
                                on_wait=[w], on_update=[])
                            out.append(nop)
                            n_new += 1
                        si.on_wait = [waits[-1]]
                        changed = True
                    out.append(inst)
                if changed:
                    blk.instructions = out

    def _drain_and_barrier(self, tick_clock, wait_clock):
        probe = self.nc.sync.nop(nofuse=True)
        wait_clock.add_sem_waits(probe.ins, ScopedClock({None: tick_clock.global_clock}))
        waits = list(probe.ins.sync_info.on_wait)
        assert self.sems is not None
        alloc = self.sems.allocated()
        by_name = {getattr(h, "name", k): h for k, h in alloc.items()}
        if len(waits) > 1:
            probe.ins.sync_info.on_wait = [waits[0]]
            for w in waits[1:]:
                self.nc.sync.wait_ge(by_name[w.ant_name], w.wait_value)
        self.nc.sync.drain()
        self.nc.all_engine_barrier()
        popped = self.nc._tile_sem_poison_stack.pop()
        assert popped is self._sem_poison
        self.nc.clear_and_free_semaphores(list(alloc.values()))
        self.nc.all_engine_barrier()


@dataclass(frozen=True)
class Cfg:
    P: int = 128          # partitions
    E: int = 1024         # embed dim
    H: int = 16           # heads
    DH: int = 64          # head dim
    HID: int = 4096       # ffn hidden
    CTX: int = 2048       # padded context length per core
    TQ: int = 512         # query tokens per core
    eps: float = 1e-5
    n_cores: int = 8
    n_vblocks: int = 1    # V/attention processed in this many head blocks
    use_bf16: bool = True

    @property
    def ET(self):
        return self.E // self.P

    @property
    def JT(self):
        return self.CTX // self.P

    @property
    def TT(self):
        return self.CTX // self.P

    @property
    def G(self):
        return self.CTX // self.TQ

    @property
    def HOT(self):
        return self.HID // self.P

    @property
    def HB(self):  # heads per V block
        return self.H // self.n_vblocks


def f32r(ap):
    return ap.bitcast(F32R)


def build_program(cfg: Cfg, split_waits: bool = True) -> bass.Bass:
    P, E, H, DH, HID = cfg.P, cfg.E, cfg.H, cfg.DH, cfg.HID
    CTX, TQ, ET, JT, G, HOT, HB, TT = (
        cfg.CTX, cfg.TQ, cfg.ET, cfg.JT, cfg.G, cfg.HOT, cfg.HB, cfg.TT)
    NVB = cfg.n_vblocks
    NG = TQ
    assert H == 2 * ET and DH * H == E and DH * 2 == P
    assert HB >= 2 and ET % NVB == 0
    VW = HB * DH  # V-projection moving width
    VH = max(1, VW // 512)   # psum output is capped at 512 fp32 columns
    VWH = VW // VH
    assert VWH <= 512

    DT = BF16 if cfg.use_bf16 else F32

    def rnd(ap):
        """Matmul-operand producer/consumer wrapper for the main datapath."""
        return ap if cfg.use_bf16 else ap.bitcast(F32R)

    nc = bass.Bass("TRN2", num_devices=cfg.n_cores)  # cachebust-v2

    xdev = nc.declare_dram_parameter("xdev", [P, ET, CTX], F32, isOutput=False)
    # pre-tiled weight chunks (host layout): contiguous per chunk
    Wqc = nc.declare_dram_parameter("Wqc", [ET, P, ET, P], DT, isOutput=False)
    Wkc = nc.declare_dram_parameter("Wkc", [ET, P, ET, P], DT, isOutput=False)
    Wvc = nc.declare_dram_parameter("Wvc", [NVB, P, ET, VW], DT, isOutput=False)
    Woc = nc.declare_dram_parameter("Woc", [ET, P, ET, P], DT, isOutput=False)
    W1c = nc.declare_dram_parameter("W1c", [HOT, P, ET, P], DT, isOutput=False)
    W2t = nc.declare_dram_parameter("W2t", [HOT, P, E], DT, isOutput=False)
    lnw1 = nc.declare_dram_parameter("lnw1", [P, ET], F32, isOutput=False)
    lnb1 = nc.declare_dram_parameter("lnb1", [P, ET], F32, isOutput=False)
    lnw2 = nc.declare_dram_parameter("lnw2", [P, ET], F32, isOutput=False)
    lnb2 = nc.declare_dram_parameter("lnb2", [P, ET], F32, isOutput=False)
    bod = nc.declare_dram_parameter("bo", [P, ET], F32, isOutput=False)
    b1d = nc.declare_dram_parameter("b1", [P, HOT], F32, isOutput=False)
    b2d = nc.declare_dram_parameter("b2", [P, ET], F32, isOutput=False)
    kbd = nc.declare_dram_parameter("kb", [P, JT], F32, isOutput=False)
    outT = nc.declare_dram_parameter("outT", [P, ET, TQ], F32, isOutput=True)

    scale = 1.0 / float(np.sqrt(DH))
    j_tri0 = (CTX - TQ) // P
    qs = slice(CTX - TQ, CTX)

    _ones_row = []  # (1, P) f32r-rounded ones, set up in the const section

    def bcast(ps_pool, tag, nparts, row):
        """Broadcast a (1, n) f32r SBUF row across nparts partitions via a
        PE outer product (ones[1,nparts].T @ row) into a PSUM tile."""
        ps_b = ps_pool.tile([nparts, row.shape[-1]], F32, tag=tag, name=f"bc_{tag}")
        nc.tensor.matmul(ps_b, f32r(_ones_row[0][:, 0:nparts]), f32r(row),
                         start=True, stop=True)
        return ps_b

    def ln_stats(rows_p, pbc_p, ps_sum, ps_sq, eps_row):
        """psum sums -> (nmean_b, rstd_b) PSUM broadcast tiles."""
        n = ps_sum.shape[-1]
        nmean = rows_p.tile([1, n], F32, tag="rows")
        nc.vector.tensor_scalar_mul(f32r(nmean), ps_sum, -1.0 / E)
        msq = rows_p.tile([1, n], F32, tag="rows")
        nc.vector.tensor_mul(msq, nmean, nmean)
        var = rows_p.tile([1, n], F32, tag="rows")
        nc.vector.scalar_tensor_tensor(
            out=var, in0=ps_sq, scalar=1.0 / E, in1=msq,
            op0=AX.mult, op1=AX.subtract)
        sq = rows_p.tile([1, n], F32, tag="rows")
        nc.scalar.activation(out=sq, in_=var, func=AF.Sqrt, bias=eps_row)
        rstd = rows_p.tile([1, n], F32, tag="rows")
        nc.vector.reciprocal(f32r(rstd), sq)
        nmean_b = bcast(pbc_p, "pbc", P, nmean)
        rstd_b = bcast(pbc_p, "pbc", P, rstd)
        return nmean_b, rstd_b

    def ln_apply(tmp_p, dst, src, nmean_b, rstd_b, w_col, b_col):
        """dst = LN(src)*w + b; intermediates in fp32, final write casts."""
        t = tmp_p.tile([P, dst.shape[-1]], F32, tag="lnt")
        nc.vector.tensor_add(t, src, nmean_b)
        nc.vector.scalar_tensor_tensor(
            out=t, in0=t, scalar=w_col, in1=rstd_b,
            op0=AX.mult, op1=AX.mult)
        nc.vector.tensor_scalar_add(rnd(dst), t, b_col)

    with TC(nc, num_cores=cfg.n_cores) as tc, \
            nc.allow_low_precision(reason="reduced-precision matmul datapath"):
        tc.do_split_waits = split_waits
        with ExitStack() as top:
            const_p = top.enter_context(tc.tile_pool(name="consts", bufs=1))
            ht_p = top.enter_context(tc.tile_pool(name="ht", bufs=1))

            ones = const_p.tile([P, 1], F32)
            nc.vector.memset(ones, 1.0)
            ones_r = const_p.tile([P, 1], F32)
            nc.vector.tensor_copy(f32r(ones_r), ones)
            ones_hb = const_p.tile([P, HB, 1], F32)
            nc.vector.memset(ones_hb, 1.0)
            ones_row = const_p.tile([1, P], F32)
            nc.vector.memset(ones_row, 1.0)
            ones_row_r = const_p.tile([1, P], F32)
            nc.vector.tensor_copy(f32r(ones_row_r), ones_row)
            _ones_row.append(ones_row_r)
            eps_row = const_p.tile([1, 1], F32)
            nc.vector.memset(eps_row, cfg.eps)
            KB = const_p.tile([P, JT], F32)
            nc.sync.dma_start(out=KB, in_=kbd[:])
            LNW1 = const_p.tile([P, ET], F32)
            nc.sync.dma_start(out=LNW1, in_=lnw1[:])
            LNB1 = const_p.tile([P, ET], F32)
            nc.sync.dma_start(out=LNB1, in_=lnb1[:])
            LNW2 = const_p.tile([P, ET], F32)
            nc.sync.dma_start(out=LNW2, in_=lnw2[:])
            LNB2 = const_p.tile([P, ET], F32)
            nc.sync.dma_start(out=LNB2, in_=lnb2[:])
            BO = const_p.tile([P, ET], F32)
            nc.sync.dma_start(out=BO, in_=bod[:])
            B1 = const_p.tile([P, HOT], F32)
            nc.sync.dma_start(out=B1, in_=b1d[:])
            B2 = const_p.tile([P, ET], F32)
            nc.sync.dma_start(out=B2, in_=b2d[:])

            # causal-triangle multiplicative masks for key tiles j_tri0..JT-1
            n_tri = JT - j_tri0
            TRI = const_p.tile([P, n_tri, TQ], DT)
            with tc.tile_pool(name="trisc", bufs=2) as tri_p:
                for jj in range(n_tri):
                    tsc = tri_p.tile([P, TQ], F32, tag="trisc")
                    nc.vector.memset(tsc, 1.0)
                    base = (CTX - TQ) - (j_tri0 + jj) * P
                    nc.gpsimd.affine_select(
                        out=tsc, in_=tsc, compare_op=AX.is_ge, fill=0.0,
                        base=base, pattern=[[1, TQ]], channel_multiplier=-1)
                    nc.vector.tensor_copy(rnd(TRI[:, jj, :]), tsc)

            with ExitStack() as mid:
                with ExitStack() as attn_sc:
                    xn_p = attn_sc.enter_context(tc.tile_pool(name="xn", bufs=1))
                    qt_p = attn_sc.enter_context(tc.tile_pool(name="qt", bufs=1))
                    va_p = attn_sc.enter_context(tc.tile_pool(name="va", bufs=1))
                    at_p = attn_sc.enter_context(tc.tile_pool(name="at", bufs=1))
                    xq_p = attn_sc.enter_context(tc.tile_pool(name="xq", bufs=1))
                    wo_p = attn_sc.enter_context(tc.tile_pool(name="wo", bufs=ET))
                    XN = xn_p.tile([P, ET, CTX], DT)
                    QT = qt_p.tile([P, ET, TQ], DT)
                    VA = va_p.tile([P, TT, HB, DH + 1], DT)
                    assert NVB == 1

                    # -------- phase A: LN1 + Q-proj + V-proj overlapped -----
                    with tc.tile_pool(name="xs", bufs=ET + 2) as xs_p, \
                         tc.tile_pool(name="xsq", bufs=3) as xsq_p, \
                         tc.tile_pool(name="lnt", bufs=3) as lnt_p, \
                         tc.tile_pool(name="rows", bufs=6) as rows_p, \
                         tc.tile_pool(name="wcq", bufs=4) as wcq_p, \
                         tc.tile_pool(name="wv", bufs=1) as wv_p, \
                         tc.tile_pool(name="pstat", bufs=3, space="PSUM") as pstat_p, \
                         tc.tile_pool(name="pbc", bufs=2, space="PSUM") as pbc_p, \
                         tc.tile_pool(name="ppv", bufs=2, space="PSUM") as ppv_p:

                        def ln_group(g):
                            gs = slice(g * NG, (g + 1) * NG)
                            ps_sum = pstat_p.tile([1, NG], F32, tag="pstat",
                                                  name=f"pssum{g}")
                            ps_sq = pstat_p.tile([1, NG], F32, tag="pstat",
                                                 name=f"pssq{g}")
                            xs_tiles = []
                            for et in range(ET):
                                xs = xs_p.tile([P, NG], F32, tag="xs")
                                nc.sync.dma_start(out=f32r(xs),
                                                  in_=f32r(xdev[:, et, gs]))
                                xs_tiles.append(xs)
                                xsq = xsq_p.tile([P, NG], F32, tag="xsq")
                                nc.scalar.square(out=f32r(xsq), in_=xs)
                                nc.tensor.matmul(ps_sum, f32r(ones_r), f32r(xs),
                                                 start=(et == 0), stop=(et == ET - 1))
                                nc.tensor.matmul(ps_sq, f32r(ones_r), f32r(xsq),
                                                 start=(et == 0), stop=(et == ET - 1))
                            nmean_b, rstd_b = ln_stats(rows_p, pbc_p, ps_sum, ps_sq,
                                                       eps_row)
                            for et in range(ET):
                                ln_apply(lnt_p, XN[:, et, gs], xs_tiles[et],
                                         nmean_b, rstd_b,
                                         LNW1[:, et:et + 1], LNB1[:, et:et + 1])

                        ln_group(G - 1)

                        # Q projection (depends only on the last group)
                        for eo in range(ET):
                            wq = wcq_p.tile([P, ET, P], DT, tag="wcq")
                            nc.sync.dma_start(out=rnd(wq), in_=rnd(Wqc[eo]))
                            ps = ppv_p.tile([P, TQ], F32, tag="ppv",
                                            name=f"psq{eo}")
                            for et in range(ET):
                                nc.tensor.matmul(ps, rnd(wq[:, et, :]),
                                                 rnd(XN[:, et, qs]),
                                                 start=(et == 0), stop=(et == ET - 1))
                            nc.vector.tensor_copy(rnd(QT[:, eo, :]), ps)

                        wv = wv_p.tile([P, ET, VW], DT)
                        nc.sync.dma_start(out=rnd(wv), in_=rnd(Wvc[0]))

                        for g in range(G - 1):
                            ln_group(g)

                        # V projection (token-major, ones column appended)
                        hh_per = VWH // DH
                        for tt in range(TT):
                            nc.vector.tensor_copy(rnd(VA[:, tt, :, DH:DH + 1]),
                                                  ones_hb)
                            for vh in range(VH):
                                ps = ppv_p.tile([P, VWH], F32, tag="ppv",
                                                name=f"psv{tt}_{vh}")
                                for et in range(ET):
                                    nc.tensor.matmul(
                                        ps, rnd(XN[:, et, tt * P:(tt + 1) * P]),
                                        rnd(wv[:, et, vh * VWH:(vh + 1) * VWH]),
                                        start=(et == 0), stop=(et == ET - 1))
                                nc.vector.tensor_copy(
                                    rnd(VA[:, tt, vh * hh_per:(vh + 1) * hh_per,
                                            0:DH]),
                                    ps.rearrange("p (h d) -> p h d", d=DH))

                    # -------- phase C: K-proj + attention + outproj --------
                    AT = at_p.tile([P, ET, TQ], DT)
                    HT = ht_p.tile([P, ET, TQ], F32)
                    XQ = xq_p.tile([P, ET, TQ], F32)
                    for et in range(ET):
                        nc.sync.dma_start(out=XQ[:, et, :], in_=xdev[:, et, qs])
                    wo_tiles = []
                    for eo in range(ET):
                        wo = wo_p.tile([P, ET, P], DT, tag="wo")
                        nc.sync.dma_start(out=rnd(wo), in_=rnd(Woc[eo]))
                        wo_tiles.append(wo)
                    with tc.tile_pool(name="wc", bufs=ET) as wc_p, \
                         tc.tile_pool(name="kt", bufs=3) as kt_p, \
                         tc.tile_pool(name="pt", bufs=4) as pt_p, \
                         tc.tile_pool(name="arow", bufs=4) as arow_p, \
                         tc.tile_pool(name="avs", bufs=4) as avs_p, \
                         tc.tile_pool(name="pproj", bufs=2, space="PSUM") as pproj_p, \
                         tc.tile_pool(name="psc", bufs=2, space="PSUM") as psc_p, \
                         tc.tile_pool(name="pav", bufs=2, space="PSUM") as pav_p:

                        pending_norm = []

                        def flush_norm():
                            while pending_norm:
                                av, h = pending_norm.pop(0)
                                rr = arow_p.tile([1, TQ], F32, tag="arow",
                                                 name=f"rr{h}")
                                nc.vector.reciprocal(f32r(rr), av[DH:DH + 1, :])
                                rb_ps = bcast(psc_p, "psc", DH, rr)
                                dst = AT[(h % 2) * DH:((h % 2) + 1) * DH,
                                         h // 2, :]
                                nc.vector.tensor_mul(rnd(dst), av[0:DH, :], rb_ps)

                        wk_tiles = []
                        for eo in range(ET):
                            wk = wc_p.tile([P, ET, P], DT, tag="wc")
                            nc.sync.dma_start(out=rnd(wk), in_=rnd(Wkc[eo]))
                            wk_tiles.append(wk)
                        for eo in range(ET):
                            hA, hB = 2 * eo, 2 * eo + 1
                            # K^T projection for heads hA, hB
                            kt = kt_p.tile([P, CTX], DT, tag="kt")
                            wk = wk_tiles[eo]
                            for g in range(G):
                                gs2 = slice(g * NG, (g + 1) * NG)
                                ps = pproj_p.tile([P, NG], F32, tag="pproj")
                                for et in range(ET):
                                    nc.tensor.matmul(
                                        ps, rnd(wk[:, et, :]), rnd(XN[:, et, gs2]),
                                        start=(et == 0), stop=(et == ET - 1))
                                nc.vector.tensor_copy(rnd(kt[:, gs2]), ps)

                            ps_avA = pav_p.tile([P, TQ], F32, tag="pav")
                            ps_avB = pav_p.tile([P, TQ], F32, tag="pav")
                            pts = {}

                            def escore(j):
                                js = slice(j * P, (j + 1) * P)
                                psc = psc_p.tile([P, 2, TQ], F32, tag="psc")
                                nc.tensor.matmul(
                                    psc[:, 0, :], rnd(kt[0:DH, js]),
                                    rnd(QT[0:DH, eo, :]), start=True, stop=True)
                                nc.tensor.matmul(
                                    psc[:, 1, :], rnd(kt[DH:P, js]),
                                    rnd(QT[DH:P, eo, :]), start=True, stop=True)
                                pt = pt_p.tile([P, 2, TQ], DT, tag="pt")
                                nc.scalar.activation(
                                    out=rnd(pt), in_=psc, func=AF.Exp,
                                    bias=KB[:, j:j + 1], scale=scale)
                                if j >= j_tri0:
                                    m = TRI[:, j - j_tri0, :]
                                    mb = bass.AP(
                                        tensor=m.tensor, offset=m.offset,
                                        ap=[list(m.ap[0]), [0, 2], list(m.ap[1])])
                                    nc.vector.tensor_mul(rnd(pt), pt, mb)
                                pts[j] = pt

                            def eav(j):
                                pt = pts.pop(j)
                                nc.tensor.matmul(
                                    ps_avA[0:DH + 1, :], rnd(VA[:, j, hA, :]),
                                    rnd(pt[:, 0, :]),
                                    start=(j == 0), stop=(j == JT - 1))
                                nc.tensor.matmul(
                                    ps_avB[0:DH + 1, :], rnd(VA[:, j, hB, :]),
                                    rnd(pt[:, 1, :]),
                                    start=(j == 0), stop=(j == JT - 1))

                            escore(0)
                            for j in range(1, JT):
                                escore(j)
                                eav(j - 1)
                            eav(JT - 1)
                            avA = avs_p.tile([DH + 1, TQ], F32, tag="avs")
                            nc.vector.tensor_copy(avA, ps_avA[0:DH + 1, :])
                            avB = avs_p.tile([DH + 1, TQ], F32, tag="avs")
                            nc.vector.tensor_copy(avB, ps_avB[0:DH + 1, :])
                            flush_norm()
                            pending_norm += [(avA, hA), (avB, hB)]
                        flush_norm()

                        # out-projection + residual -> HT
                        for eo in range(ET):
                            ps = pproj_p.tile([P, TQ], F32, tag="pproj",
                                              name=f"pso{eo}")
                            for et in range(ET):
                                nc.tensor.matmul(ps, rnd(wo_tiles[eo][:, et, :]),
                                                 rnd(AT[:, et, :]),
                                                 start=(et == 0), stop=(et == ET - 1))
                            dst = HT[:, eo, :]
                            nc.vector.tensor_add(f32r(dst), ps, XQ[:, eo, :])
                            nc.vector.tensor_scalar_add(f32r(dst), dst,
                                                        BO[:, eo:eo + 1])

                # -------- LN2 --------
                lt_p = mid.enter_context(tc.tile_pool(name="lt", bufs=1))
                rt_p = mid.enter_context(tc.tile_pool(name="rt", bufs=1))
                LT = lt_p.tile([P, ET, TQ], DT)
                RT = rt_p.tile([P, HOT, TQ], DT)
                with tc.tile_pool(name="lnt2", bufs=3) as lnt2_p, \
                     tc.tile_pool(name="sq2", bufs=3) as sq2_p, \
                     tc.tile_pool(name="rows2", bufs=6) as rows2_p, \
                     tc.tile_pool(name="pstat2", bufs=2, space="PSUM") as pstat2_p, \
                     tc.tile_pool(name="pbc2", bufs=2, space="PSUM") as pbc2_p:
                    ps_sum = pstat2_p.tile([1, TQ], F32, tag="pstat2", name="l2sum")
                    ps_sq = pstat2_p.tile([1, TQ], F32, tag="pstat2", name="l2sq")
                    for et in range(ET):
                        hsq = sq2_p.tile([P, TQ], F32, tag="sq2")
                        nc.scalar.square(out=f32r(hsq), in_=HT[:, et, :])
                        nc.tensor.matmul(ps_sum, f32r(ones_r),
                                         f32r(HT[:, et, :]),
                                         start=(et == 0), stop=(et == ET - 1))
                        nc.tensor.matmul(ps_sq, f32r(ones_r), f32r(hsq),
                                         start=(et == 0), stop=(et == ET - 1))
                    nmean_b, rstd_b = ln_stats(rows2_p, pbc2_p, ps_sum, ps_sq,
                                               eps_row)
                    for et in range(ET):
                        ln_apply(lnt2_p, LT[:, et, :], HT[:, et, :],
                                 nmean_b, rstd_b,
                                 LNW2[:, et:et + 1], LNB2[:, et:et + 1])

                # -------- FFN1 + FFN2 first half (pipelined per ho) --------
                EH = ET // 2
                w2br_p = mid.enter_context(tc.tile_pool(name="w2br", bufs=1))
                W2BR = w2br_p.tile([P, HOT, E - EH * P], DT)
                with tc.tile_pool(name="w1", bufs=6) as w1_p, \
                     tc.tile_pool(name="w2a", bufs=4) as w2a_p, \
                     tc.tile_pool(name="ot", bufs=3) as ot_p, \
                     tc.tile_pool(name="pf2a", bufs=EH, space="PSUM") as pf2a_p:
                    pf1_ctx = ExitStack()
                    pf1_p = pf1_ctx.enter_context(
                        tc.tile_pool(name="pf1", bufs=3, space="PSUM"))
                    ps8a = [pf2a_p.tile([P, TQ], F32, tag="pf2a", name=f"ps8a_{i}")
                            for i in range(EH)]
                    def effn1(ho):
                        w1s = w1_p.tile([P, ET, P], DT, tag="w1")
                        nc.sync.dma_start(out=rnd(w1s), in_=rnd(W1c[ho]))
                        ps = pf1_p.tile([P, TQ], F32, tag="pf1", name=f"psf{ho}")
                        for et in range(ET):
                            nc.tensor.matmul(ps, rnd(w1s[:, et, :]),
                                             rnd(LT[:, et, :]),
                                             start=(et == 0), stop=(et == ET - 1))
                        nc.scalar.activation(out=rnd(RT[:, ho, :]), in_=ps,
                                             func=AF.Relu, bias=B1[:, ho:ho + 1])

                    def effn2a(ho):
                        nc.sync.dma_start(out=rnd(W2BR[:, ho, :]),
                                          in_=rnd(W2t[ho, :, EH * P:E]))
                        w2a = w2a_p.tile([P, EH * P], DT, tag="w2a")
                        nc.sync.dma_start(out=rnd(w2a),
                                          in_=rnd(W2t[ho, :, 0:EH * P]))
                        for eo in range(EH):
                            nc.tensor.matmul(
                                ps8a[eo], rnd(w2a[:, eo * P:(eo + 1) * P]),
                                rnd(RT[:, ho, :]),
                                start=(ho == 0), stop=(ho == HOT - 1))

                    effn1(0)
                    for ho in range(1, HOT):
                        effn1(ho)
                        effn2a(ho - 1)
                    effn2a(HOT - 1)
                    pf1_ctx.close()

                    # -------- FFN2 second half (pf1 banks recycled) --------
                    with tc.tile_pool(name="ot2", bufs=3) as ot2_p, \
                         tc.tile_pool(name="pf2b", bufs=ET - EH,
                                      space="PSUM") as pf2b_p:
                        ps8b = [pf2b_p.tile([P, TQ], F32, tag="pf2b",
                                            name=f"ps8b_{i}")
                                for i in range(ET - EH)]
                        first = True
                        for ho in range(HOT):
                            for eo in range(EH, ET):
                                nc.tensor.matmul(
                                    ps8b[eo - EH],
                                    rnd(W2BR[:, ho,
                                             (eo - EH) * P:(eo - EH + 1) * P]),
                                    rnd(RT[:, ho, :]),
                                    start=(ho == 0), stop=(ho == HOT - 1))
                            if first:
                                # drain first-half outputs behind 2b matmuls
                                first = False
                                for eo in range(EH):
                                    o = ot_p.tile([P, TQ], F32, tag="ot")
                                    nc.vector.tensor_add(o, ps8a[eo],
                                                         HT[:, eo, :])
                                    nc.vector.tensor_scalar_add(
                                        o, o, B2[:, eo:eo + 1])
                                    nc.sync.dma_start(out=outT[:, eo, :],
                                                      in_=o)
                        for eo in range(EH, ET):
                            o = ot2_p.tile([P, TQ], F32, tag="ot2")
                            nc.vector.tensor_add(o, ps8b[eo - EH],
                                                 HT[:, eo, :])
                            nc.vector.tensor_scalar_add(o, o, B2[:, eo:eo + 1])
                            nc.sync.dma_start(out=outT[:, eo, :], in_=o)
    return nc


# ------------------------- host side -------------------------

def _np_dt(cfg: Cfg):
    if cfg.use_bf16:
        import ml_dtypes
        return ml_dtypes.bfloat16
    return np.float32


def make_weight_inputs(cfg: Cfg, Wq, Wk, Wv, Wo, W1, W2):
    """Pre-tile weights into contiguous chunk layouts (shared by all cores)."""
    P, E, HID, ET, HOT, NVB, HB, DH = (
        cfg.P, cfg.E, cfg.HID, cfg.ET, cfg.HOT, cfg.n_vblocks, cfg.HB, cfg.DH)
    VW = HB * DH
    dt = _np_dt(cfg)

    def chunks_pp(W):  # (E,E) -> (ET_out, P, ET_in, P) slabs
        W = np.asarray(W, dtype=np.float32)
        # slab[eo, p, et, j] = W[et*P+p, eo*P+j]
        c = W.reshape(ET, P, ET, P).transpose(2, 1, 0, 3)
        return np.ascontiguousarray(c.astype(dt))

    Wv = np.asarray(Wv, dtype=np.float32)
    wvc = Wv.reshape(ET, P, NVB, VW).transpose(2, 1, 0, 3)
    W1 = np.asarray(W1, dtype=np.float32)
    w1c = W1.reshape(ET, P, HOT, P).transpose(2, 1, 0, 3)
    W2 = np.asarray(W2, dtype=np.float32)
    w2t = W2.reshape(HOT, P, E)

    return {
        "Wqc": chunks_pp(Wq),
        "Wkc": chunks_pp(Wk),
        "Woc": chunks_pp(Wo),
        "Wvc": np.ascontiguousarray(wvc.astype(dt)),
        "W1c": np.ascontiguousarray(w1c.astype(dt)),
        "W2t": np.ascontiguousarray(w2t.astype(dt)),
    }


def make_core_inputs(cfg: Cfg, core: int, weight_inputs, x, bo, ln1_w, ln1_b,
                     ln2_w, ln2_b, b1, b2):
    P, E, CTX, TQ, ET, JT, HOT = (
        cfg.P, cfg.E, cfg.CTX, cfg.TQ, cfg.ET, cfg.JT, cfg.HOT)
    n_chunks = CTX // TQ
    b, ci = core // n_chunks, core % n_chunks
    ctx_len = (ci + 1) * TQ
    pad = CTX - ctx_len

    xT = np.zeros((E, CTX), dtype=np.float32)
    xT[:, pad:] = np.asarray(x[b, :ctx_len], dtype=np.float32).T
    xdev = np.ascontiguousarray(xT.reshape(ET, P, CTX).transpose(1, 0, 2))

    kb = np.where(np.arange(CTX) < pad, np.float32(MASK_NEG), np.float32(0.0))
    kb = (kb + np.float32(EXP_SHIFT)).astype(np.float32)
    kb2d = np.ascontiguousarray(kb.reshape(JT, P).T)

    def cols(v, nt):
        return np.ascontiguousarray(
            np.asarray(v, dtype=np.float32).reshape(nt, P).T)

    m = {
        "xdev": xdev,
        "lnw1": cols(ln1_w, ET), "lnb1": cols(ln1_b, ET),
        "lnw2": cols(ln2_w, ET), "lnb2": cols(ln2_b, ET),
        "bo": cols(bo, ET), "b1": cols(b1, HOT), "b2": cols(b2, ET),
        "kb": kb2d,
    }
    m.update(weight_inputs)
    return m


def make_all_core_inputs(cfg: Cfg, **inputs):
    w = make_weight_inputs(cfg, inputs["Wq"], inputs["Wk"], inputs["Wv"],
                           inputs["Wo"], inputs["W1"], inputs["W2"])
    rest = {k: inputs[k] for k in
            ("x", "bo", "ln1_w", "ln1_b", "ln2_w", "ln2_b", "b1", "b2")}
    return [make_core_inputs(cfg, c, w, **rest) for c in range(cfg.n_cores)]


def unshard_output(cfg: Cfg, results):
    """results: list of per-core dicts with 'outT' -> full (B, S, E)."""
    P, E, TQ, ET, CTX = cfg.P, cfg.E, cfg.TQ, cfg.ET, cfg.CTX
    n_chunks = CTX // TQ
    B = cfg.n_cores // n_chunks
    S = n_chunks * TQ
    out = np.empty((B, S, E), dtype=np.float32)
    for core in range(cfg.n_cores):
        b, ci = core // n_chunks, core % n_chunks
        oT = results[core]["outT"]  # (P, ET, TQ)
        out[b, ci * TQ:(ci + 1) * TQ, :] = (
            oT.transpose(1, 0, 2).reshape(E, TQ).T)
    return out


_CACHE = {}


def _get_program(cfg: Cfg) -> bass.Bass:
    if cfg not in _CACHE:
        _CACHE[cfg] = build_program(cfg)
    return _CACHE[cfg]


def kernel(**inputs) -> np.ndarray:
    from concourse.bass_utils import run_bass_kernel_spmd
    cfg = Cfg()
    nc = _get_program(cfg)
    in_maps = make_all_core_inputs(cfg, **inputs)
    res = run_bass_kernel_spmd(nc, in_maps, list(range(cfg.n_cores)))
    return unshard_output(cfg, res.results)



# revision 46
# speedup vs baseline: 1.2265x; 1.0275x over previous
"""Trainium2 Bass kernel for a dense transformer block (pre-LN, causal MHA, FFN).

Sharding: sequence-parallel over 8 cores. Each batch (B=2) is split into 4
query chunks of 512 tokens; core c handles batch c//4, chunk c%4. Each core
recomputes LN1+K/V over its causal context. To keep one SPMD program across
cores, the context is FRONT-PADDED with zeros to 2048 tokens so the query
chunk always sits at local positions [1536, 2048) -- the causal triangle is
then core-invariant, and padded keys are masked via a per-key additive bias
folded into the softmax Exp's per-partition bias operand (scores are computed
transposed, keys on partitions, so a per-key bias is a per-partition scalar).

Layout: activations are feature-major (E on partitions) so weight matrices
(stored (E_in, E_out)) serve directly as matmul lhsT tiles; weight chunks are
pre-tiled on the host so every chunk DMA is a single contiguous burst.
Transposed scores feed the P@V matmul without any transposes; the softmax
denominator comes from a ones column appended to V's lhsT (row DH of the
attnV psum). Softmax skips max-subtraction (scores are O(1) by construction;
a constant -5 shift is folded into the key bias).

Precision: the matmul datapath runs bf16 x bf16 -> fp32-psum (use_bf16=True;
fast weight loads + halved weight traffic). LN statistics, softmax
denominators, residuals and the final output stay fp32 (stats matmuls run as
float32r). With use_bf16=False the whole datapath runs float32r instead.
"""

import sys
from contextlib import ExitStack
from dataclasses import dataclass

import numpy as np

if "/opt/trn_rl_repo" not in sys.path:
    sys.path.insert(0, "/opt/trn_rl_repo")

import concourse.bass as bass  # noqa: E402
import concourse.mybir as mybir  # noqa: E402
import concourse.tile as tile  # noqa: E402
from concourse.vector_clock import ScopedClock  # noqa: E402

F32 = mybir.dt.float32
F32R = mybir.dt.float32r
BF16 = mybir.dt.bfloat16
AX = mybir.AluOpType
AF = mybir.ActivationFunctionType

MASK_NEG = -30000.0
EXP_SHIFT = -5.0


class TC(tile.TileContext):
    """TileContext whose kernel-tail drain splits its sem waits across
    separate SP instructions -- walrus in this env rejects >2 sync waits
    on one CTRL-class instruction -- and which post-splits any multi-wait
    instruction (the S3_LW fp32 matmul struct tolerates only one sync
    wait) by hoisting extra waits onto same-engine NoOps."""

    do_split_waits = True  # disable for CoreSim (breaks its fake-update bookkeeping)

    def schedule_and_allocate(self, *a, **k):
        ret = super().schedule_and_allocate(*a, **k)
        if self.do_split_waits:
            self._split_multiwaits()
        return ret

    def _split_multiwaits(self):
        import bass_rust
        n_new = 0
        for fn in self.nc.m.functions:
            for blk in fn.blocks:
                insts = list(blk.instructions)
                out = []
                changed = False
                for inst in insts:
                    si = inst.sync_info
                    waits = list(si.on_wait) if si is not None else []
                    if len(waits) > 1:
                        for w in waits[:-1]:
                            nop = mybir.InstNoOp(
                                name=f"{inst.name}-sw{n_new}", ins=[], outs=[])
                            nop.engine = inst.engine
                            nop.sync_info = bass_rust.SyncInfo(
                                on_wait=[w], on_update=[])
                            out.append(nop)
                            n_new += 1
                        si.on_wait = [waits[-1]]
                        changed = True
                    out.append(inst)
                if changed:
                    blk.instructions = out

    def _drain_and_barrier(self, tick_clock, wait_clock):
        probe = self.nc.sync.nop(nofuse=True)
        wait_clock.add_sem_waits(probe.ins, ScopedClock({None: tick_clock.global_clock}))
        waits = list(probe.ins.sync_info.on_wait)
        assert self.sems is not None
        alloc = self.sems.allocated()
        by_name = {getattr(h, "name", k): h for k, h in alloc.items()}
        if len(waits) > 1:
            probe.ins.sync_info.on_wait = [waits[0]]
            for w in waits[1:]:
                self.nc.sync.wait_ge(by_name[w.ant_name], w.wait_value)
        self.nc.sync.drain()
        self.nc.all_engine_barrier()
        popped = self.nc._tile_sem_poison_stack.pop()
        assert popped is self._sem_poison
        self.nc.clear_and_free_semaphores(list(alloc.values()))
        self.nc.all_engine_barrier()


@dataclass(frozen=True)
class Cfg:
    P: int = 128          # partitions
    E: int = 1024         # embed dim
    H: int = 16           # heads
    DH: int = 64          # head dim
    HID: int = 4096       # ffn hidden
    CTX: int = 2048       # padded context length per core
    TQ: int = 512         # query tokens per core
    eps: float = 1e-5
    n_cores: int = 8
    n_vblocks: int = 1    # V/attention processed in this many head blocks
    use_bf16: bool = True

    @property
    def ET(self):
        return self.E // self.P

    @property
    def JT(self):
        return self.CTX // self.P

    @property
    def TT(self):
        return self.CTX // self.P

    @property
    def G(self):
        return self.CTX // self.TQ

    @property
    def HOT(self):
        return self.HID // self.P

    @property
    def HB(self):  # heads per V block
        return self.H // self.n_vblocks


def f32r(ap):
    return ap.bitcast(F32R)


def build_program(cfg: Cfg, split_waits: bool = True) -> bass.Bass:
    P, E, H, DH, HID = cfg.P, cfg.E, cfg.H, cfg.DH, cfg.HID
    CTX, TQ, ET, JT, G, HOT, HB, TT = (
        cfg.CTX, cfg.TQ, cfg.ET, cfg.JT, cfg.G, cfg.HOT, cfg.HB, cfg.TT)
    NVB = cfg.n_vblocks
    NG = TQ
    assert H == 2 * ET and DH * H == E and DH * 2 == P
    assert HB >= 2 and ET % NVB == 0
    VW = HB * DH  # V-projection moving width
    VH = max(1, VW // 512)   # psum output is capped at 512 fp32 columns
    VWH = VW // VH
    assert VWH <= 512

    DT = BF16 if cfg.use_bf16 else F32

    def rnd(ap):
        """Matmul-operand producer/consumer wrapper for the main datapath."""
        return ap if cfg.use_bf16 else ap.bitcast(F32R)

    nc = bass.Bass("TRN2", num_devices=cfg.n_cores)  # cachebust-v2

    xdev = nc.declare_dram_parameter("xdev", [P, ET, CTX], F32, isOutput=False)
    # pre-tiled weight chunks (host layout): contiguous per chunk
    Wqc = nc.declare_dram_parameter("Wqc", [ET, P, ET, P], DT, isOutput=False)
    Wkc = nc.declare_dram_parameter("Wkc", [ET, P, ET, P], DT, isOutput=False)
    Wvc = nc.declare_dram_parameter("Wvc", [NVB, P, ET, VW], DT, isOutput=False)
    Woc = nc.declare_dram_parameter("Woc", [ET, P, ET, P], DT, isOutput=False)
    W1c = nc.declare_dram_parameter("W1c", [HOT, P, ET, P], DT, isOutput=False)
    W2t = nc.declare_dram_parameter("W2t", [HOT, P, E], DT, isOutput=False)
    lnw1 = nc.declare_dram_parameter("lnw1", [P, ET], F32, isOutput=False)
    lnb1 = nc.declare_dram_parameter("lnb1", [P, ET], F32, isOutput=False)
    lnw2 = nc.declare_dram_parameter("lnw2", [P, ET], F32, isOutput=False)
    lnb2 = nc.declare_dram_parameter("lnb2", [P, ET], F32, isOutput=False)
    bod = nc.declare_dram_parameter("bo", [P, ET], F32, isOutput=False)
    b1d = nc.declare_dram_parameter("b1", [P, HOT], F32, isOutput=False)
    b2d = nc.declare_dram_parameter("b2", [P, ET], F32, isOutput=False)
    kbd = nc.declare_dram_parameter("kb", [P, JT], F32, isOutput=False)
    outT = nc.declare_dram_parameter("outT", [P, ET, TQ], F32, isOutput=True)

    scale = 1.0 / float(np.sqrt(DH))
    j_tri0 = (CTX - TQ) // P
    qs = slice(CTX - TQ, CTX)

    _ones_row = []  # (1, P) f32r-rounded ones, set up in the const section

    def bcast(ps_pool, tag, nparts, row):
        """Broadcast a (1, n) f32r SBUF row across nparts partitions via a
        PE outer product (ones[1,nparts].T @ row) into a PSUM tile."""
        ps_b = ps_pool.tile([nparts, row.shape[-1]], F32, tag=tag, name=f"bc_{tag}")
        nc.tensor.matmul(ps_b, f32r(_ones_row[0][:, 0:nparts]), f32r(row),
                         start=True, stop=True)
        return ps_b

    def ln_stats(rows_p, pbc_p, ps_sum, ps_sq, eps_row):
        """psum sums -> (nmean_b, rstd_b) PSUM broadcast tiles."""
        n = ps_sum.shape[-1]
        nmean = rows_p.tile([1, n], F32, tag="rows")
        nc.vector.tensor_scalar_mul(f32r(nmean), ps_sum, -1.0 / E)
        msq = rows_p.tile([1, n], F32, tag="rows")
        nc.vector.tensor_mul(msq, nmean, nmean)
        var = rows_p.tile([1, n], F32, tag="rows")
        nc.vector.scalar_tensor_tensor(
            out=var, in0=ps_sq, scalar=1.0 / E, in1=msq,
            op0=AX.mult, op1=AX.subtract)
        lnv = rows_p.tile([1, n], F32, tag="rows")
        nc.scalar.activation(out=lnv, in_=var, func=AF.Ln, bias=eps_row)
        rste = rows_p.tile([1, n], F32, tag="rows")
        nc.scalar.activation(out=rste, in_=lnv, func=AF.Exp, scale=-0.5)
        rstd = rows_p.tile([1, n], F32, tag="rows")
        nc.vector.tensor_copy(f32r(rstd), rste)
        nmean_b = bcast(pbc_p, "pbc", P, nmean)
        rstd_b = bcast(pbc_p, "pbc", P, rstd)
        return nmean_b, rstd_b

    def ln_apply(tmp_p, dst, src, nmean_b, rstd_b, w_col, b_col):
        """dst = LN(src)*w + b; intermediates in fp32, final write casts."""
        t = tmp_p.tile([P, dst.shape[-1]], F32, tag="lnt")
        nc.vector.tensor_add(t, src, nmean_b)
        nc.vector.scalar_tensor_tensor(
            out=t, in0=t, scalar=w_col, in1=rstd_b,
            op0=AX.mult, op1=AX.mult)
        nc.vector.tensor_scalar_add(rnd(dst), t, b_col)

    with TC(nc, num_cores=cfg.n_cores) as tc, \
            nc.allow_low_precision(reason="reduced-precision matmul datapath"):
        tc.do_split_waits = split_waits
        with ExitStack() as top:
            const_p = top.enter_context(tc.tile_pool(name="consts", bufs=1))
            ht_p = top.enter_context(tc.tile_pool(name="ht", bufs=1))

            ones = const_p.tile([P, 1], F32)
            nc.vector.memset(ones, 1.0)
            ones_r = const_p.tile([P, 1], F32)
            nc.vector.tensor_copy(f32r(ones_r), ones)
            ones_hb = const_p.tile([P, HB, 1], F32)
            nc.vector.memset(ones_hb, 1.0)
            ones_row = const_p.tile([1, P], F32)
            nc.vector.memset(ones_row, 1.0)
            ones_row_r = const_p.tile([1, P], F32)
            nc.vector.tensor_copy(f32r(ones_row_r), ones_row)
            _ones_row.append(ones_row_r)
            eps_row = const_p.tile([1, 1], F32)
            nc.vector.memset(eps_row, cfg.eps)
            KB = const_p.tile([P, JT], F32)
            nc.sync.dma_start(out=KB, in_=kbd[:])
            LNW1 = const_p.tile([P, ET], F32)
            nc.sync.dma_start(out=LNW1, in_=lnw1[:])
            LNB1 = const_p.tile([P, ET], F32)
            nc.sync.dma_start(out=LNB1, in_=lnb1[:])
            LNW2 = const_p.tile([P, ET], F32)
            nc.sync.dma_start(out=LNW2, in_=lnw2[:])
            LNB2 = const_p.tile([P, ET], F32)
            nc.sync.dma_start(out=LNB2, in_=lnb2[:])
            BO = const_p.tile([P, ET], F32)
            nc.sync.dma_start(out=BO, in_=bod[:])
            B1 = const_p.tile([P, HOT], F32)
            nc.sync.dma_start(out=B1, in_=b1d[:])
            B2 = const_p.tile([P, ET], F32)
            nc.sync.dma_start(out=B2, in_=b2d[:])

            # causal-triangle multiplicative masks for key tiles j_tri0..JT-1
            n_tri = JT - j_tri0
            TRI = const_p.tile([P, n_tri, TQ], DT)
            with tc.tile_pool(name="trisc", bufs=2) as tri_p:
                for jj in range(n_tri):
                    tsc = tri_p.tile([P, TQ], F32, tag="trisc")
                    nc.vector.memset(tsc, 1.0)
                    base = (CTX - TQ) - (j_tri0 + jj) * P
                    nc.gpsimd.affine_select(
                        out=tsc, in_=tsc, compare_op=AX.is_ge, fill=0.0,
                        base=base, pattern=[[1, TQ]], channel_multiplier=-1)
                    nc.vector.tensor_copy(rnd(TRI[:, jj, :]), tsc)

            with ExitStack() as mid:
                with ExitStack() as attn_sc:
                    xn_p = attn_sc.enter_context(tc.tile_pool(name="xn", bufs=1))
                    qt_p = attn_sc.enter_context(tc.tile_pool(name="qt", bufs=1))
                    va_p = attn_sc.enter_context(tc.tile_pool(name="va", bufs=1))
                    at_p = attn_sc.enter_context(tc.tile_pool(name="at", bufs=1))
                    xq_p = attn_sc.enter_context(tc.tile_pool(name="xq", bufs=1))
                    wo_p = attn_sc.enter_context(tc.tile_pool(name="wo", bufs=ET))
                    XN = xn_p.tile([P, ET, CTX], DT)
                    QT = qt_p.tile([P, ET, TQ], DT)
                    VA = va_p.tile([P, TT, HB, DH + 1], DT)
                    assert NVB == 1

                    # -------- phase A: LN1 + Q-proj + V-proj overlapped -----
                    with tc.tile_pool(name="xs", bufs=ET + 2) as xs_p, \
                         tc.tile_pool(name="xsq", bufs=3) as xsq_p, \
                         tc.tile_pool(name="lnt", bufs=3) as lnt_p, \
                         tc.tile_pool(name="rows", bufs=6) as rows_p, \
                         tc.tile_pool(name="wcq", bufs=4) as wcq_p, \
                         tc.tile_pool(name="wv", bufs=1) as wv_p, \
                         tc.tile_pool(name="pstat", bufs=3, space="PSUM") as pstat_p, \
                         tc.tile_pool(name="pbc", bufs=2, space="PSUM") as pbc_p, \
                         tc.tile_pool(name="ppv", bufs=2, space="PSUM") as ppv_p:

                        def ln_group(g):
                            gs = slice(g * NG, (g + 1) * NG)
                            ps_sum = pstat_p.tile([1, NG], F32, tag="pstat",
                                                  name=f"pssum{g}")
                            ps_sq = pstat_p.tile([1, NG], F32, tag="pstat",
                                                 name=f"pssq{g}")
                            xs_tiles = []
                            for et in range(ET):
                                xs = xs_p.tile([P, NG], F32, tag="xs")
                                nc.sync.dma_start(out=f32r(xs),
                                                  in_=f32r(xdev[:, et, gs]))
                                xs_tiles.append(xs)
                                xsq = xsq_p.tile([P, NG], F32, tag="xsq")
                                nc.scalar.square(out=f32r(xsq), in_=xs)
                                nc.tensor.matmul(ps_sum, f32r(ones_r), f32r(xs),
                                                 start=(et == 0), stop=(et == ET - 1))
                                nc.tensor.matmul(ps_sq, f32r(ones_r), f32r(xsq),
                                                 start=(et == 0), stop=(et == ET - 1))
                            nmean_b, rstd_b = ln_stats(rows_p, pbc_p, ps_sum, ps_sq,
                                                       eps_row)
                            for et in range(ET):
                                ln_apply(lnt_p, XN[:, et, gs], xs_tiles[et],
                                         nmean_b, rstd_b,
                                         LNW1[:, et:et + 1], LNB1[:, et:et + 1])

                        ln_group(G - 1)

                        # Q projection (depends only on the last group)
                        for eo in range(ET):
                            wq = wcq_p.tile([P, ET, P], DT, tag="wcq")
                            nc.sync.dma_start(out=rnd(wq), in_=rnd(Wqc[eo]))
                            ps = ppv_p.tile([P, TQ], F32, tag="ppv",
                                            name=f"psq{eo}")
                            for et in range(ET):
                                nc.tensor.matmul(ps, rnd(wq[:, et, :]),
                                                 rnd(XN[:, et, qs]),
                                                 start=(et == 0), stop=(et == ET - 1))
                            nc.vector.tensor_copy(rnd(QT[:, eo, :]), ps)

                        wv = wv_p.tile([P, ET, VW], DT)
                        nc.sync.dma_start(out=rnd(wv), in_=rnd(Wvc[0]))

                        for g in range(G - 1):
                            ln_group(g)

                        # V projection (token-major, ones column appended)
                        hh_per = VWH // DH
                        for tt in range(TT):
                            nc.vector.tensor_copy(rnd(VA[:, tt, :, DH:DH + 1]),
                                                  ones_hb)
                            for vh in range(VH):
                                ps = ppv_p.tile([P, VWH], F32, tag="ppv",
                                                name=f"psv{tt}_{vh}")
                                for et in range(ET):
                                    nc.tensor.matmul(
                                        ps, rnd(XN[:, et, tt * P:(tt + 1) * P]),
                                        rnd(wv[:, et, vh * VWH:(vh + 1) * VWH]),
                                        start=(et == 0), stop=(et == ET - 1))
                                nc.vector.tensor_copy(
                                    rnd(VA[:, tt, vh * hh_per:(vh + 1) * hh_per,
                                            0:DH]),
                                    ps.rearrange("p (h d) -> p h d", d=DH))

                    # -------- phase C: K-proj + attention + outproj --------
                    AT = at_p.tile([P, ET, TQ], DT)
                    HT = ht_p.tile([P, ET, TQ], F32)
                    XQ = xq_p.tile([P, ET, TQ], F32)
                    for et in range(ET):
                        nc.sync.dma_start(out=XQ[:, et, :], in_=xdev[:, et, qs])
                    wo_tiles = []
                    for eo in range(ET):
                        wo = wo_p.tile([P, ET, P], DT, tag="wo")
                        nc.sync.dma_start(out=rnd(wo), in_=rnd(Woc[eo]))
                        wo_tiles.append(wo)
                    with tc.tile_pool(name="wc", bufs=ET) as wc_p, \
                         tc.tile_pool(name="kt", bufs=3) as kt_p, \
                         tc.tile_pool(name="pt", bufs=4) as pt_p, \
                         tc.tile_pool(name="arow", bufs=4) as arow_p, \
                         tc.tile_pool(name="avs", bufs=4) as avs_p, \
                         tc.tile_pool(name="pproj", bufs=2, space="PSUM") as pproj_p, \
                         tc.tile_pool(name="psc", bufs=2, space="PSUM") as psc_p, \
                         tc.tile_pool(name="pav", bufs=2, space="PSUM") as pav_p:

                        pending_norm = []

                        def flush_norm():
                            while pending_norm:
                                av, h = pending_norm.pop(0)
                                lnd = arow_p.tile([1, TQ], F32, tag="arow",
                                                  name=f"ld{h}")
                                nc.scalar.activation(out=lnd,
                                                     in_=av[DH:DH + 1, :],
                                                     func=AF.Ln)
                                rre = arow_p.tile([1, TQ], F32, tag="arow",
                                                  name=f"re{h}")
                                nc.scalar.activation(out=rre, in_=lnd,
                                                     func=AF.Exp, scale=-1.0)
                                rr = arow_p.tile([1, TQ], F32, tag="arow",
                                                 name=f"rr{h}")
                                nc.vector.tensor_copy(f32r(rr), rre)
                                rb_ps = bcast(psc_p, "psc", DH, rr)
                                dst = AT[(h % 2) * DH:((h % 2) + 1) * DH,
                                         h // 2, :]
                                nc.vector.tensor_mul(rnd(dst), av[0:DH, :], rb_ps)

                        wk_tiles = []
                        for eo in range(ET):
                            wk = wc_p.tile([P, ET, P], DT, tag="wc")
                            nc.sync.dma_start(out=rnd(wk), in_=rnd(Wkc[eo]))
                            wk_tiles.append(wk)
                        for eo in range(ET):
                            hA, hB = 2 * eo, 2 * eo + 1
                            # K^T projection for heads hA, hB
                            kt = kt_p.tile([P, CTX], DT, tag="kt")
                            wk = wk_tiles[eo]
                            for g in range(G):
                                gs2 = slice(g * NG, (g + 1) * NG)
                                ps = pproj_p.tile([P, NG], F32, tag="pproj")
                                for et in range(ET):
                                    nc.tensor.matmul(
                                        ps, rnd(wk[:, et, :]), rnd(XN[:, et, gs2]),
                                        start=(et == 0), stop=(et == ET - 1))
                                nc.vector.tensor_copy(rnd(kt[:, gs2]), ps)

                            ps_avA = pav_p.tile([P, TQ], F32, tag="pav")
                            ps_avB = pav_p.tile([P, TQ], F32, tag="pav")
                            pts = {}

                            def escore(j):
                                js = slice(j * P, (j + 1) * P)
                                psc = psc_p.tile([P, 2, TQ], F32, tag="psc")
                                nc.tensor.matmul(
                                    psc[:, 0, :], rnd(kt[0:DH, js]),
                                    rnd(QT[0:DH, eo, :]), start=True, stop=True)
                                nc.tensor.matmul(
                                    psc[:, 1, :], rnd(kt[DH:P, js]),
                                    rnd(QT[DH:P, eo, :]), start=True, stop=True)
                                pt = pt_p.tile([P, 2, TQ], DT, tag="pt")
                                nc.scalar.activation(
                                    out=rnd(pt), in_=psc, func=AF.Exp,
                                    bias=KB[:, j:j + 1], scale=scale)
                                if j >= j_tri0:
                                    m = TRI[:, j - j_tri0, :]
                                    mb = bass.AP(
                                        tensor=m.tensor, offset=m.offset,
                                        ap=[list(m.ap[0]), [0, 2], list(m.ap[1])])
                                    nc.vector.tensor_mul(rnd(pt), pt, mb)
                                pts[j] = pt

                            def eav(j):
                                pt = pts.pop(j)
                                nc.tensor.matmul(
                                    ps_avA[0:DH + 1, :], rnd(VA[:, j, hA, :]),
                                    rnd(pt[:, 0, :]),
                                    start=(j == 0), stop=(j == JT - 1))
                                nc.tensor.matmul(
                                    ps_avB[0:DH + 1, :], rnd(VA[:, j, hB, :]),
                                    rnd(pt[:, 1, :]),
                                    start=(j == 0), stop=(j == JT - 1))

                            escore(0)
                            for j in range(1, JT):
                                escore(j)
                                eav(j - 1)
                            eav(JT - 1)
                            avA = avs_p.tile([DH + 1, TQ], F32, tag="avs")
                            nc.vector.tensor_copy(avA, ps_avA[0:DH + 1, :])
                            avB = avs_p.tile([DH + 1, TQ], F32, tag="avs")
                            nc.vector.tensor_copy(avB, ps_avB[0:DH + 1, :])
                            flush_norm()
                            pending_norm += [(avA, hA), (avB, hB)]
                        flush_norm()

                        # out-projection + residual -> HT
                        for eo in range(ET):
                            ps = pproj_p.tile([P, TQ], F32, tag="pproj",
                                              name=f"pso{eo}")
                            for et in range(ET):
                                nc.tensor.matmul(ps, rnd(wo_tiles[eo][:, et, :]),
                                                 rnd(AT[:, et, :]),
                                                 start=(et == 0), stop=(et == ET - 1))
                            dst = HT[:, eo, :]
                            nc.vector.tensor_add(f32r(dst), ps, XQ[:, eo, :])
                            nc.vector.tensor_scalar_add(f32r(dst), dst,
                                                        BO[:, eo:eo + 1])

                # -------- LN2 --------
                lt_p = mid.enter_context(tc.tile_pool(name="lt", bufs=1))
                rt_p = mid.enter_context(tc.tile_pool(name="rt", bufs=1))
                LT = lt_p.tile([P, ET, TQ], DT)
                RT = rt_p.tile([P, HOT, TQ], DT)
                with tc.tile_pool(name="lnt2", bufs=3) as lnt2_p, \
                     tc.tile_pool(name="sq2", bufs=3) as sq2_p, \
                     tc.tile_pool(name="rows2", bufs=6) as rows2_p, \
                     tc.tile_pool(name="pstat2", bufs=2, space="PSUM") as pstat2_p, \
                     tc.tile_pool(name="pbc2", bufs=2, space="PSUM") as pbc2_p:
                    ps_sum = pstat2_p.tile([1, TQ], F32, tag="pstat2", name="l2sum")
                    ps_sq = pstat2_p.tile([1, TQ], F32, tag="pstat2", name="l2sq")
                    for et in range(ET):
                        hsq = sq2_p.tile([P, TQ], F32, tag="sq2")
                        nc.scalar.square(out=f32r(hsq), in_=HT[:, et, :])
                        nc.tensor.matmul(ps_sum, f32r(ones_r),
                                         f32r(HT[:, et, :]),
                                         start=(et == 0), stop=(et == ET - 1))
                        nc.tensor.matmul(ps_sq, f32r(ones_r), f32r(hsq),
                                         start=(et == 0), stop=(et == ET - 1))
                    nmean_b, rstd_b = ln_stats(rows2_p, pbc2_p, ps_sum, ps_sq,
                                               eps_row)
                    for et in range(ET):
                        ln_apply(lnt2_p, LT[:, et, :], HT[:, et, :],
                                 nmean_b, rstd_b,
                                 LNW2[:, et:et + 1], LNB2[:, et:et + 1])

                # -------- FFN1 + FFN2 first half (pipelined per ho) --------
                EH = ET // 2
                w2br_p = mid.enter_context(tc.tile_pool(name="w2br", bufs=1))
                W2BR = w2br_p.tile([P, HOT, E - EH * P], DT)
                with tc.tile_pool(name="w1", bufs=6) as w1_p, \
                     tc.tile_pool(name="w2a", bufs=4) as w2a_p, \
                     tc.tile_pool(name="ot", bufs=3) as ot_p, \
                     tc.tile_pool(name="pf2a", bufs=EH, space="PSUM") as pf2a_p:
                    pf1_ctx = ExitStack()
                    pf1_p = pf1_ctx.enter_context(
                        tc.tile_pool(name="pf1", bufs=3, space="PSUM"))
                    ps8a = [pf2a_p.tile([P, TQ], F32, tag="pf2a", name=f"ps8a_{i}")
                            for i in range(EH)]
                    def effn1(ho):
                        w1s = w1_p.tile([P, ET, P], DT, tag="w1")
                        nc.sync.dma_start(out=rnd(w1s), in_=rnd(W1c[ho]))
                        ps = pf1_p.tile([P, TQ], F32, tag="pf1", name=f"psf{ho}")
                        for et in range(ET):
                            nc.tensor.matmul(ps, rnd(w1s[:, et, :]),
                                             rnd(LT[:, et, :]),
                                             start=(et == 0), stop=(et == ET - 1))
                        nc.scalar.activation(out=rnd(RT[:, ho, :]), in_=ps,
                                             func=AF.Relu, bias=B1[:, ho:ho + 1])

                    def effn2a(ho):
                        nc.sync.dma_start(out=rnd(W2BR[:, ho, :]),
                                          in_=rnd(W2t[ho, :, EH * P:E]))
                        w2a = w2a_p.tile([P, EH * P], DT, tag="w2a")
                        nc.sync.dma_start(out=rnd(w2a),
                                          in_=rnd(W2t[ho, :, 0:EH * P]))
                        for eo in range(EH):
                            nc.tensor.matmul(
                                ps8a[eo], rnd(w2a[:, eo * P:(eo + 1) * P]),
                                rnd(RT[:, ho, :]),
                                start=(ho == 0), stop=(ho == HOT - 1))

                    effn1(0)
                    for ho in range(1, HOT):
                        effn1(ho)
                        effn2a(ho - 1)
                    effn2a(HOT - 1)
                    pf1_ctx.close()

                    # -------- FFN2 second half (pf1 banks recycled) --------
                    with tc.tile_pool(name="ot2", bufs=3) as ot2_p, \
                         tc.tile_pool(name="pf2b", bufs=ET - EH,
                                      space="PSUM") as pf2b_p:
                        ps8b = [pf2b_p.tile([P, TQ], F32, tag="pf2b",
                                            name=f"ps8b_{i}")
                                for i in range(ET - EH)]
                        first = True
                        for ho in range(HOT):
                            for eo in range(EH, ET):
                                nc.tensor.matmul(
                                    ps8b[eo - EH],
                                    rnd(W2BR[:, ho,
                                             (eo - EH) * P:(eo - EH + 1) * P]),
                                    rnd(RT[:, ho, :]),
                                    start=(ho == 0), stop=(ho == HOT - 1))
                            if first:
                                # drain first-half outputs behind 2b matmuls
                                first = False
                                for eo in range(EH):
                                    o = ot_p.tile([P, TQ], F32, tag="ot")
                                    nc.vector.tensor_add(o, ps8a[eo],
                                                         HT[:, eo, :])
                                    nc.vector.tensor_scalar_add(
                                        o, o, B2[:, eo:eo + 1])
                                    nc.sync.dma_start(out=outT[:, eo, :],
                                                      in_=o)
                        for eo in range(EH, ET):
                            o = ot2_p.tile([P, TQ], F32, tag="ot2")
                            nc.vector.tensor_add(o, ps8b[eo - EH],
                                                 HT[:, eo, :])
                            nc.vector.tensor_scalar_add(o, o, B2[:, eo:eo + 1])
                            nc.sync.dma_start(out=outT[:, eo, :], in_=o)
    return nc


# ------------------------- host side -------------------------

def _np_dt(cfg: Cfg):
    if cfg.use_bf16:
        import ml_dtypes
        return ml_dtypes.bfloat16
    return np.float32


def make_weight_inputs(cfg: Cfg, Wq, Wk, Wv, Wo, W1, W2):
    """Pre-tile weights into contiguous chunk layouts (shared by all cores)."""
    P, E, HID, ET, HOT, NVB, HB, DH = (
        cfg.P, cfg.E, cfg.HID, cfg.ET, cfg.HOT, cfg.n_vblocks, cfg.HB, cfg.DH)
    VW = HB * DH
    dt = _np_dt(cfg)

    def chunks_pp(W):  # (E,E) -> (ET_out, P, ET_in, P) slabs
        W = np.asarray(W, dtype=np.float32)
        # slab[eo, p, et, j] = W[et*P+p, eo*P+j]
        c = W.reshape(ET, P, ET, P).transpose(2, 1, 0, 3)
        return np.ascontiguousarray(c.astype(dt))

    Wv = np.asarray(Wv, dtype=np.float32)
    wvc = Wv.reshape(ET, P, NVB, VW).transpose(2, 1, 0, 3)
    W1 = np.asarray(W1, dtype=np.float32)
    w1c = W1.reshape(ET, P, HOT, P).transpose(2, 1, 0, 3)
    W2 = np.asarray(W2, dtype=np.float32)
    w2t = W2.reshape(HOT, P, E)

    return {
        "Wqc": chunks_pp(Wq),
        "Wkc": chunks_pp(Wk),
        "Woc": chunks_pp(Wo),
        "Wvc": np.ascontiguousarray(wvc.astype(dt)),
        "W1c": np.ascontiguousarray(w1c.astype(dt)),
        "W2t": np.ascontiguousarray(w2t.astype(dt)),
    }


def make_core_inputs(cfg: Cfg, core: int, weight_inputs, x, bo, ln1_w, ln1_b,
                     ln2_w, ln2_b, b1, b2):
    P, E, CTX, TQ, ET, JT, HOT = (
        cfg.P, cfg.E, cfg.CTX, cfg.TQ, cfg.ET, cfg.JT, cfg.HOT)
    n_chunks = CTX // TQ
    b, ci = core // n_chunks, core % n_chunks
    ctx_len = (ci + 1) * TQ
    pad = CTX - ctx_len

    xT = np.zeros((E, CTX), dtype=np.float32)
    xT[:, pad:] = np.asarray(x[b, :ctx_len], dtype=np.float32).T
    xdev = np.ascontiguousarray(xT.reshape(ET, P, CTX).transpose(1, 0, 2))

    kb = np.where(np.arange(CTX) < pad, np.float32(MASK_NEG), np.float32(0.0))
    kb = (kb + np.float32(EXP_SHIFT)).astype(np.float32)
    kb2d = np.ascontiguousarray(kb.reshape(JT, P).T)

    def cols(v, nt):
        return np.ascontiguousarray(
            np.asarray(v, dtype=np.float32).reshape(nt, P).T)

    m = {
        "xdev": xdev,
        "lnw1": cols(ln1_w, ET), "lnb1": cols(ln1_b, ET),
        "lnw2": cols(ln2_w, ET), "lnb2": cols(ln2_b, ET),
        "bo": cols(bo, ET), "b1": cols(b1, HOT), "b2": cols(b2, ET),
        "kb": kb2d,
    }
    m.update(weight_inputs)
    return m


def make_all_core_inputs(cfg: Cfg, **inputs):
    w = make_weight_inputs(cfg, inputs["Wq"], inputs["Wk"], inputs["Wv"],
                           inputs["Wo"], inputs["W1"], inputs["W2"])
    rest = {k: inputs[k] for k in
            ("x", "bo", "ln1_w", "ln1_b", "ln2_w", "ln2_b", "b1", "b2")}
    return [make_core_inputs(cfg, c, w, **rest) for c in range(cfg.n_cores)]


def unshard_output(cfg: Cfg, results):
    """results: list of per-core dicts with 'outT' -> full (B, S, E)."""
    P, E, TQ, ET, CTX = cfg.P, cfg.E, cfg.TQ, cfg.ET, cfg.CTX
    n_chunks = CTX // TQ
    B = cfg.n_cores // n_chunks
    S = n_chunks * TQ
    out = np.empty((B, S, E), dtype=np.float32)
    for core in range(cfg.n_cores):
        b, ci = core // n_chunks, core % n_chunks
        oT = results[core]["outT"]  # (P, ET, TQ)
        out[b, ci * TQ:(ci + 1) * TQ, :] = (
            oT.transpose(1, 0, 2).reshape(E, TQ).T)
    return out


_CACHE = {}


def _get_program(cfg: Cfg) -> bass.Bass:
    if cfg not in _CACHE:
        _CACHE[cfg] = build_program(cfg)
    return _CACHE[cfg]


def kernel(**inputs) -> np.ndarray:
    from concourse.bass_utils import run_bass_kernel_spmd
    cfg = Cfg()
    nc = _get_program(cfg)
    in_maps = make_all_core_inputs(cfg, **inputs)
    res = run_bass_kernel_spmd(nc, in_maps, list(range(cfg.n_cores)))
    return unshard_output(cfg, res.results)

